# revision 1
# baseline (speedup 1.0000x reference)
"""DeepseekV2 MoE layer on 8 TRN2 NeuronCores (expert-parallel).

Sharding: w1/w2 sharded 4-experts-per-core; gate + token activations
replicated; shared expert tensor-parallel along the FS dim (352/core,
zero-padded to 384). Routing (softmax + grouped top-k) computed on device.
Each core computes its 4 experts' contributions for all tokens via
gather -> MLP -> weighted one-hot combine, plus its shared-expert slice,
into a partial [T, H]; a ReduceScatter sums partials and each core emits
output token rows [128k : 128(k+1)); the host concatenates.

Device-dtype choices: router matmul fp32 (top-k selection must match the
fp32 reference ordering); expert/shared matmuls bf16 (weights host-cast);
combine matmul fp32r; cumsum/slot matmuls exact (0/1 bf16 / small-int f32).
"""

import numpy as np
import ml_dtypes

import concourse.bass as bass
import concourse.tile as tile
from concourse import bacc, mybir
from concourse.bass import ds
from concourse.masks import make_identity
from concourse.tile_rust import add_dep_helper

# problem shape
T, H = 1024, 2048
E, F = 32, 1408
F2 = 2 * F                      # 2816
G_GRP, TOPK_G, TOPK = 8, 3, 6
FS = 2 * F                      # 2816 shared intermediate
SCALE = 16.0
NCORES = 8
EL = E // NCORES                # 4 experts per core
C = 256                         # per-expert token capacity (max seen ~214)
P = 128
TT = T // P                     # 8 token tiles
HC = H // P                     # 16 h chunks
FT = F // P                     # 11 f tiles
F2T = F2 // P                   # 22
SSH = 384                       # padded shared shard (352 real)

F32 = mybir.dt.float32
F32R = mybir.dt.float32r
BF16 = mybir.dt.bfloat16
I32 = mybir.dt.int32
AF = mybir.ActivationFunctionType
OP = mybir.AluOpType


DEBUG = False


def build_program():
    nc = bacc.Bacc("TRN2", target_bir_lowering=False, debug=False,
                   num_devices=NCORES)

    xT_d = nc.dram_tensor("xT", [H, T], F32, kind="ExternalInput")
    x_d = nc.dram_tensor("x", [T, H], F32, kind="ExternalInput")
    wgT_d = nc.dram_tensor("wgT", [H, E], F32, kind="ExternalInput")
    w1_d = nc.dram_tensor("w1l", [EL, H, F2], BF16, kind="ExternalInput")
    w2_d = nc.dram_tensor("w2l", [EL, F, H], BF16, kind="ExternalInput")
    ws1_d = nc.dram_tensor("ws1l", [H, 2 * SSH], BF16, kind="ExternalInput")
    ws2_d = nc.dram_tensor("ws2l", [SSH, H], BF16, kind="ExternalInput")
    sel_d = nc.dram_tensor("sel", [E, EL], F32, kind="ExternalInput")
    out_d = nc.dram_tensor("out", [P, H], F32, kind="ExternalOutput")

    acc_d = nc.dram_tensor("acc_d", [T, H], F32)
    rs_d = nc.dram_tensor("rs_d", [P, H], F32)
    dbg = {}
    if DEBUG:
        dbg["logT"] = nc.dram_tensor("d_logT", [E, T], F32, kind="ExternalOutput")
        dbg["scores"] = nc.dram_tensor("d_scores", [P, TT * E], F32, kind="ExternalOutput")
        dbg["comb"] = nc.dram_tensor("d_comb", [P, TT * E], F32, kind="ExternalOutput")
        dbg["pos"] = nc.dram_tensor("d_pos", [E, T], F32, kind="ExternalOutput")
        dbg["combT"] = nc.dram_tensor("d_combT", [E, T], F32, kind="ExternalOutput")
        dbg["srow"] = nc.dram_tensor("d_srow", [P, T], F32, kind="ExternalOutput")
        dbg["slotcol"] = nc.dram_tensor("d_slotcol", [P, TT], F32, kind="ExternalOutput")
        dbg["stok"] = nc.dram_tensor("d_stok", [P, 2], I32, kind="ExternalOutput")
        dbg["xet"] = nc.dram_tensor("d_xet", [P, C], BF16, kind="ExternalOutput")
        dbg["act"] = nc.dram_tensor("d_act", [P, C], BF16, kind="ExternalOutput")
        dbg["y"] = nc.dram_tensor("d_y", [P, 512], F32, kind="ExternalOutput")
        dbg["gmat"] = nc.dram_tensor("d_gmat", [P, T], F32, kind="ExternalOutput")
        dbg["accs"] = nc.dram_tensor("d_accs", [P, H], F32, kind="ExternalOutput")

    dmas = []
    ccs = []

    with tile.TileContext(nc) as tc:
        _build(nc, tc, locals())

    nc.compile()
    return nc


def _build(nc, tc, env):
    g = env
    xT_d, x_d, wgT_d = g["xT_d"], g["x_d"], g["wgT_d"]
    w1_d, w2_d, ws1_d, ws2_d = g["w1_d"], g["w2_d"], g["ws1_d"], g["ws2_d"]
    out_d, acc_d, rs_d, sel_d = g["out_d"], g["acc_d"], g["rs_d"], g["sel_d"]
    dbg = g["dbg"]
    dmas, ccs = g["dmas"], g["ccs"]

    def dma(*a, **k):
        r = nc.gpsimd.dma_start(*a, **k)
        dmas.append(r)
        return r

    def idma(*a, **k):
        r = nc.gpsimd.indirect_dma_start(*a, **k)
        dmas.append(r)
        return r

    import contextlib
    ctx = contextlib.ExitStack()
    sb = ctx.enter_context(tc.tile_pool(name="sb", bufs=1))
    sb_xt = ctx.enter_context(tc.tile_pool(name="sb_xt", bufs=2))
    sb_w1 = ctx.enter_context(tc.tile_pool(name="sb_w1", bufs=2))
    sb_w2 = ctx.enter_context(tc.tile_pool(name="sb_w2", bufs=2))
    sb_ws1 = ctx.enter_context(tc.tile_pool(name="sb_ws1", bufs=3))
    sb_xe = ctx.enter_context(tc.tile_pool(name="sb_xe", bufs=2))
    sb_rot = ctx.enter_context(tc.tile_pool(name="sb_rot", bufs=1))
    sb_xet = ctx.enter_context(tc.tile_pool(name="sb_xet", bufs=1))
    sb_et = ctx.enter_context(tc.tile_pool(name="sb_et", bufs=3))
    ps_a = ctx.enter_context(tc.tile_pool(name="ps_a", bufs=4, space="PSUM"))
    ps_b = ctx.enter_context(tc.tile_pool(name="ps_b", bufs=2, space="PSUM"))
    ps_tr = ctx.enter_context(tc.tile_pool(name="ps_tr", bufs=2, space="PSUM"))

    # ---- constants ----
    ident = sb.tile([P, P], F32)
    make_identity(nc, ident[:])
    iota_c_row_i = sb.tile([P, C], I32)
    nc.gpsimd.iota(iota_c_row_i[:], pattern=[[1, C]], base=0, channel_multiplier=0)
    iota_c_row = sb.tile([P, C], F32)
    nc.vector.tensor_copy(iota_c_row[:], iota_c_row_i[:])
    iota_half_i = sb.tile([P, 2], I32)   # col h: value 128*h + p
    nc.gpsimd.iota(iota_half_i[:], pattern=[[P, 2]], base=0, channel_multiplier=1)
    iota_half = sb.tile([P, 2], F32)
    nc.vector.tensor_copy(iota_half[:], iota_half_i[:])
    tok_iota_i = sb.tile([P, TT], I32)   # col k: value 128*k + p
    nc.gpsimd.iota(tok_iota_i[:], pattern=[[P, TT]], base=0, channel_multiplier=1)
    tok_iota = sb.tile([P, TT], F32)
    nc.vector.tensor_copy(tok_iota[:], tok_iota_i[:])
    ones_bf = sb.tile([P, T // 2], BF16)
    nc.vector.memset(ones_bf[:], 1.0)
    ones_row = sb.tile([1, P], F32)
    nc.vector.memset(ones_row[:], 1.0)

    # ---- stage R1: router logitsT + shared-expert gate/up pass ----
    wg_sb = sb.tile([P, HC * E], F32)
    dma(out=wg_sb[:].rearrange("p (c e) -> p c e", e=E),
        in_=wgT_d[:, :].rearrange("(c p) e -> p c e", p=P))


    logT_sb = sb_et.tile([E, T], F32, tag="et", name="logT_sb")
    # shared gate/up accumulate: 6 m-tiles of 128 rows (gate 0..2, up 3..5)
    # passes: (mgrp in 3) x (n in 2) with 2 m-tiles each -> psum 2 live
    act_sT = sb.tile([P, 3 * T], BF16)

    # router psum [32, 512] x2 sequential
    for n in range(2):
        ps_l = ps_b.tile([E, T // 2], F32, tag="big", name=f"psl{n}")
        for k in range(HC):
            xt = sb_xt.tile([P, T], F32, tag="xt")
            if n == 0:
                dma(out=xt[:], in_=xT_d[k * P:(k + 1) * P, :])
            else:
                dma(out=xt[:, T // 2:], in_=xT_d[k * P:(k + 1) * P, T // 2:])
            nc.tensor.matmul(
                ps_l[:], wg_sb[:, k * E:(k + 1) * E],
                xt[:, n * (T // 2):(n + 1) * (T // 2)],
                start=(k == 0), stop=(k == HC - 1))
        nc.vector.tensor_copy(logT_sb[:, n * (T // 2):(n + 1) * (T // 2)], ps_l[:])

    # shared expert MM_s1: lhsT = ws1l [H, 768] chunks; rhs = xT
    # loop: for mg in 3: for n in 2: psum[2] over k in 16
    for mg in range(3):
        for n in range(2):
            psg = ps_b.tile([P, T // 2], F32, tag="big", name=f"psg{mg}{n}")
            psu = ps_b.tile([P, T // 2], F32, tag="big", name=f"psu{mg}{n}")
            for k in range(HC):
                ws1t = sb_ws1.tile([P, 2 * P], BF16, tag="ws1")
                # gate m-tile mg cols [mg*128, +128); up cols [384 + mg*128, +128)
                dma(out=ws1t[:, :P],
                    in_=ws1_d[k * P:(k + 1) * P, mg * P:(mg + 1) * P])
                dma(out=ws1t[:, P:],
                    in_=ws1_d[k * P:(k + 1) * P, SSH + mg * P:SSH + (mg + 1) * P])
                xt = sb_xt.tile([P, T], F32, tag="xt")
                dma(out=xt[:, n * (T // 2):(n + 1) * (T // 2)],
                    in_=xT_d[k * P:(k + 1) * P, n * (T // 2):(n + 1) * (T // 2)])
                xbf = sb_xt.tile([P, T // 2], BF16, tag="xbf")
                nc.vector.tensor_copy(
                    xbf[:], xt[:, n * (T // 2):(n + 1) * (T // 2)])
                nc.tensor.matmul(psg[:], ws1t[:, :P], xbf[:],
                                 start=(k == 0), stop=(k == HC - 1))
                nc.tensor.matmul(psu[:], ws1t[:, P:], xbf[:],
                                 start=(k == 0), stop=(k == HC - 1))
            sl = n * (T // 2)
            gsil = sb_rot.tile([P, T // 2], F32, tag="gsil")
            nc.scalar.activation(gsil[:], psg[:], AF.Sigmoid)
            nc.vector.tensor_tensor(out=gsil[:], in0=gsil[:], in1=psg[:],
                                    op=OP.mult)
            nc.vector.tensor_tensor(
                out=act_sT[:, mg * T + sl:mg * T + sl + T // 2],
                in0=gsil[:], in1=psu[:], op=OP.mult)

    # ---- stage R2: routing math ----
    # transpose logitsT -> logits [128, 8*32]
    scores = sb.tile([P, TT * E], F32)
    for k in range(TT):
        pst = ps_tr.tile([P, P], F32, tag="tr")
        nc.tensor.transpose(pst[:, :E], logT_sb[:, k * P:(k + 1) * P],
                            ident[:E, :E])
        nc.vector.tensor_copy(scores[:, k * E:(k + 1) * E], pst[:, :E])

    # softmax per token row over 32 experts (per t-tile)
    tmp8 = sb.tile([P, 8], F32)
    for k in range(TT):
        blk = scores[:, k * E:(k + 1) * E]
        mx = sb.tile([P, 1], F32, tag="rmax", name=f"rmax{k}")
        nc.vector.tensor_reduce(mx[:], blk, axis=mybir.AxisListType.X,
                                op=OP.max, negate=True)
        sm = sb.tile([P, 1], F32, tag="rsum", name=f"rsum{k}")
        nc.scalar.activation(blk, blk, AF.Exp, bias=mx[:], accum_out=sm[:])
        rc = sb.tile([P, 1], F32, tag="rrec", name=f"rrec{k}")
        nc.vector.reciprocal(rc[:], sm[:])
        nc.vector.tensor_scalar_mul(blk, blk, rc[:])

    comb = sb.tile([P, TT * E], F32)
    mask_bf = sb.tile([P, TT * E], BF16)
    for k in range(TT):
        blk = scores[:, k * E:(k + 1) * E]
        blk3 = scores[:, k * E:(k + 1) * E].rearrange("p (g f) -> p g f", f=4)
        gsc = sb.tile([P, G_GRP], F32, tag="gsc", name=f"gsc{k}")
        nc.vector.tensor_reduce(gsc[:], blk3, axis=mybir.AxisListType.X, op=OP.max)
        nc.vector.max(out=tmp8[:], in_=gsc[:])
        nc.vector.memset(tmp8[:, TOPK_G:], 0.0)
        gz = sb.tile([P, G_GRP], F32, tag="gz", name=f"gz{k}")
        nc.vector.match_replace(out=gz[:], in_to_replace=tmp8[:],
                                in_values=gsc[:], imm_value=0.0)
        # gmask = (gsc - gz) > 0
        nc.vector.tensor_tensor(out=gz[:], in0=gsc[:], in1=gz[:], op=OP.subtract)
        nc.vector.tensor_scalar(gz[:], gz[:], 0.0, scalar2=None, op0=OP.is_gt)
        # masked = scores * repeat(gmask, 4)
        cblk = comb[:, k * E:(k + 1) * E]
        cblk3 = comb[:, k * E:(k + 1) * E].rearrange("p (g f) -> p g f", f=4)
        gz3 = gz[:].rearrange("p (g o) -> p g o", o=1)
        nc.vector.tensor_tensor(out=cblk3, in0=blk3,
                                in1=gz3.to_broadcast([P, G_GRP, 4]), op=OP.mult)
        # top-6 of masked
        nc.vector.max(out=tmp8[:], in_=cblk)
        nc.vector.memset(tmp8[:, TOPK:], 0.0)
        zap = sb.tile([P, E], F32, tag="zap", name=f"zap{k}")
        nc.vector.match_replace(out=zap[:], in_to_replace=tmp8[:],
                                in_values=cblk, imm_value=0.0)
        nc.vector.tensor_tensor(out=cblk, in0=cblk, in1=zap[:], op=OP.subtract)
        nc.vector.tensor_scalar_mul(cblk, cblk, SCALE)
        nc.vector.tensor_copy(mask_bf[:, k * E:(k + 1) * E], cblk)
        nc.vector.tensor_scalar(mask_bf[:, k * E:(k + 1) * E],
                                mask_bf[:, k * E:(k + 1) * E],
                                0.0, scalar2=None, op0=OP.is_gt)

    if DEBUG:
        dma(out=dbg["scores"][:, :], in_=scores[:])
        dma(out=dbg["comb"][:, :], in_=comb[:])
    # transpose comb -> combT [32, 1024]
    combT = sb_et.tile([E, T], F32, tag="et", name="combT")
    for k in range(TT):
        pst = ps_tr.tile([P, P], F32, tag="tr")
        nc.tensor.transpose(pst[:E, :P], comb[:, k * E:(k + 1) * E], ident[:])
        nc.vector.tensor_copy(combT[:, k * P:(k + 1) * P], pst[:E, :P])

    # cumsum over tokens: pos[e, t] = sum_{t'<=t} mask[e, t']
    pos = sb_et.tile([E, T], F32, tag="et", name="pos")
    for n in range(2):
        psc = ps_b.tile([E, T // 2], F32, tag="big", name=f"psc{n}")
        for k in range(TT):
            lk = sb_rot.tile([P, T // 2], BF16, tag="lk")
            nc.gpsimd.affine_select(
                out=lk[:], in_=ones_bf[:], pattern=[[1, T // 2]],
                compare_op=OP.is_ge, fill=0.0,
                base=n * (T // 2) - k * P, channel_multiplier=-1)
            nc.tensor.matmul(psc[:], mask_bf[:, k * E:(k + 1) * E], lk[:],
                             start=(k == 0), stop=(k == TT - 1))
        nc.vector.tensor_copy(pos[:, n * (T // 2):(n + 1) * (T // 2)], psc[:])

    # slot32[e, t] = mask ? pos-1 : C   (clamped to C), computed in place:
    # slot32 = (pos - 1 - C) * mask + C ; then clamp to C
    maskT = sb_et.tile([E, T], F32, tag="et", name="maskT")
    nc.vector.tensor_scalar(maskT[:], combT[:], 0.0, scalar2=None, op0=OP.is_gt)
    nc.vector.tensor_scalar(pos[:], pos[:], float(1 + C), scalar2=None,
                            op0=OP.subtract)
    nc.vector.tensor_tensor(out=pos[:], in0=pos[:], in1=maskT[:], op=OP.mult)
    nc.vector.tensor_scalar(pos[:], pos[:], float(C), scalar2=None, op0=OP.add)
    nc.vector.tensor_scalar_min(pos[:], pos[:], float(C))

    if DEBUG:
        dma(out=dbg["logT"][:, :], in_=logT_sb[:])
        dma(out=dbg["pos"][:, :], in_=pos[:])
        dma(out=dbg["combT"][:, :], in_=combT[:])
    sel_sb = sb.tile([E, EL], F32)
    dma(out=sel_sb[:], in_=sel_d[:, :])

    # ---- output accumulator ----
    acc = sb.tile([P, TT * H], F32)   # [t-tile-major: (k, h)] 64KB/partition

    # shared expert MM_s2: y_s[t, h] = act_s^T.T @ ws2   (lhsT = act_sT chunks)
    for n in range(4):
        ws2t = sb_w2.tile([P, 3 * 512], BF16, tag="ws2", name=f"ws2_{n}")
        dma(out=ws2t[:].rearrange("p (c h) -> p c h", h=512),
            in_=ws2_d[:, n * 512:(n + 1) * 512].rearrange(
                "(c p) h -> p c h", p=P))
        for mt in range(TT):
            psy = ps_b.tile([P, 512], F32, tag="big", name=f"psys{n}{mt}")
            for kf in range(3):
                nc.tensor.matmul(
                    psy[:],
                    act_sT[:, kf * T + mt * P:kf * T + (mt + 1) * P],
                    ws2t[:, kf * 512:(kf + 1) * 512],
                    start=(kf == 0), stop=(kf == 2))
            nc.vector.tensor_copy(acc[:, mt * H + n * 512:mt * H + (n + 1) * 512],
                                  psy[:])

    # ---- experts ----
    for e in range(EL):
        # select expert row (4*pid + e) of slot32/combT and broadcast to all
        # partitions: psum = sel128.T @ rows  (sel is a per-core one-hot input)
        sel128 = sb_rot.tile([E, P], F32, tag="sel128")
        nc.vector.tensor_copy(sel128[:], sel_sb[:, e:e + 1].to_broadcast([E, P]))
        srow = sb_rot.tile([P, T], F32, tag="srow")
        crow = sb_rot.tile([P, T], F32, tag="crow")
        for src, dst in ((pos, srow), (combT, crow)):
            for nn in range(2):
                psb = ps_b.tile([P, 512], F32, tag="big",
                                name=f"bc_{e}_{dst.name}_{nn}")
                nc.tensor.matmul(psb[:], sel128[:],
                                 src[:, nn * 512:(nn + 1) * 512],
                                 start=True, stop=True)
                nc.vector.tensor_copy(dst[:, nn * 512:(nn + 1) * 512], psb[:])
        # slot values in [128(t), 8] layout: PE-transpose srow chunks (all
        # partitions of srow are equal, so any column of the transpose works)
        slotcol = sb_rot.tile([P, TT], F32, tag="slotcol")
        for k in range(TT):
            pst = ps_tr.tile([P, P], F32, tag="tr", name=f"sc_{e}_{k}")
            nc.tensor.transpose(pst[:], srow[:, k * P:(k + 1) * P], ident[:])
            nc.vector.tensor_copy(slotcol[:, k:k + 1], pst[:, 0:1])

        # slot_tokens[c] = sum_t (slot[t] == c) * t   (exact fp32 matmul)
        stok = sb_rot.tile([P, 2], I32, tag="stok")
        for half in range(2):
            pss = ps_tr.tile([P, P], F32, tag="tr")
            for k in range(TT):
                petk = sb_rot.tile([P, P], F32, tag="petk")
                nc.vector.tensor_tensor(
                    out=petk[:],
                    in0=slotcol[:, k:k + 1].to_broadcast([P, P]),
                    in1=iota_c_row[:, half * P:(half + 1) * P], op=OP.is_equal)
                nc.tensor.matmul(
                    pss[:, :1], petk[:], tok_iota[:, k:k + 1],
                    start=(k == 0), stop=(k == TT - 1))
            nc.vector.tensor_copy(stok[:, half:half + 1], pss[:, :1])

        if DEBUG and e == 0:
            dma(out=dbg["srow"][:, :], in_=srow[:])
            dma(out=dbg["slotcol"][:, :], in_=slotcol[:])
            dma(out=dbg["stok"][:, :], in_=stok[:])
        # gather X_e rows [C, H] then transpose to XeT [h-chunks, 256]
        xet = sb_xet.tile([P, HC * C], BF16, tag="xet")
        for half in range(2):
            xe = sb_xe.tile([P, H], F32, tag="xe")
            idma(out=xe[:], out_offset=None, in_=x_d[:, :],
                 in_offset=bass.IndirectOffsetOnAxis(
                     ap=stok[:, half:half + 1], axis=0))
            for hc in range(HC):
                pst = ps_tr.tile([P, P], F32, tag="tr")
                nc.tensor.transpose(pst[:], xe[:, hc * P:(hc + 1) * P], ident[:])
                nc.vector.tensor_copy(
                    xet[:, hc * C + half * P:hc * C + half * P + P], pst[:])

        if DEBUG and e == 0:
            dma(out=dbg["xet"][:, :], in_=xet[:, 0:C])
        # MM1: gu^T tiles = w1^T @ XeT ; 4 column-passes of <=6 m-tiles
        gate_e = sb.tile([P, FT * C], BF16, tag="gate_e")
        act_e = sb.tile([P, FT * C], BF16, tag="act_e")
        groups = [list(range(0, 4)), list(range(4, 8)), list(range(8, 11)),
                  list(range(11, 15)), list(range(15, 19)), list(range(19, 22))]
        for gi, grp in enumerate(groups):
            pss = [ps_a.tile([P, C], F32, tag="mm1", name=f"mm1_{e}_{gi}_{j}")
                   for j in range(len(grp))]
            w0 = grp[0] * P
            wn = (grp[-1] + 1) * P - w0
            for k in range(HC):
                w1t = sb_w1.tile([P, 4 * P], BF16, tag="w1")
                dma(out=w1t[:, :wn], in_=w1_d[e, k * P:(k + 1) * P, w0:w0 + wn])
                for j, m in enumerate(grp):
                    nc.tensor.matmul(pss[j][:], w1t[:, j * P:j * P + P],
                                     xet[:, k * C:(k + 1) * C],
                                     start=(k == 0), stop=(k == HC - 1))
            for j, m in enumerate(grp):
                if m < FT:  # gate tile: silu(gate) -> bf16
                    sgt = sb_rot.tile([P, C], F32, tag="sgt",
                                      name=f"sgt_{e}_{m}")
                    nc.scalar.activation(sgt[:], pss[j][:], AF.Sigmoid)
                    nc.vector.tensor_tensor(out=gate_e[:, m * C:(m + 1) * C],
                                            in0=sgt[:], in1=pss[j][:],
                                            op=OP.mult)
                else:       # up tile: act = silu(gate) * up -> bf16
                    mm = m - FT
                    nc.vector.tensor_tensor(
                        out=act_e[:, mm * C:(mm + 1) * C],
                        in0=gate_e[:, mm * C:(mm + 1) * C],
                        in1=pss[j][:], op=OP.mult)

        # MM2': y [c-tile, H] = act^T.T @ w2 ; 4 column passes (512 cols)
        y_sb = sb.tile([P, 2 * H], F32R, tag="y_sb")
        for n in range(4):
            psy = [ps_b.tile([P, 512], F32, tag="big", name=f"y_{e}_{n}_{j}")
                   for j in range(2)]
            for kf in range(FT):
                w2t = sb_w2.tile([P, 512], BF16, tag="w2")
                dma(out=w2t[:],
                    in_=w2_d[e, kf * P:(kf + 1) * P, n * 512:(n + 1) * 512])
                for mc in range(2):
                    nc.tensor.matmul(
                        psy[mc][:],
                        act_e[:, kf * C + mc * P:kf * C + mc * P + P],
                        w2t[:], start=(kf == 0), stop=(kf == FT - 1))
            for mc in range(2):
                nc.vector.tensor_copy(
                    y_sb[:, mc * H + n * 512:mc * H + (n + 1) * 512], psy[mc][:])

        if DEBUG and e == 0:
            dma(out=dbg["act"][:, :], in_=act_e[:, 0:C])
            dma(out=dbg["y"][:, :], in_=y_sb[:, 0:512])
        # G matrices [c-half, T] = (iota_col == slot_row) * combT_row
        gmat = sb.tile([P, 2 * T], F32R, tag="gmat")
        for half in range(2):
            nc.vector.tensor_tensor(
                out=gmat[:, half * T:(half + 1) * T],
                in0=iota_half[:, half:half + 1].to_broadcast([P, T]),
                in1=srow[:], op=OP.is_equal)
            nc.vector.tensor_tensor(
                out=gmat[:, half * T:(half + 1) * T],
                in0=gmat[:, half * T:(half + 1) * T],
                in1=crow[:], op=OP.mult)

        if DEBUG and e == 0:
            dma(out=dbg["gmat"][:, :], in_=gmat[:, 0:T])
        # combine: acc[t-tile, h] += G^T @ y
        for mt in range(TT):
            for n in range(4):
                pso = ps_b.tile([P, 512], F32, tag="big", name=f"o_{e}_{mt}_{n}")
                for half in range(2):
                    nc.tensor.matmul(
                        pso[:],
                        gmat[:, half * T + mt * P:half * T + (mt + 1) * P],
                        y_sb[:, half * H + n * 512:half * H + (n + 1) * 512],
                        start=(half == 0), stop=(half == 1))
                nc.vector.tensor_tensor(
                    out=acc[:, mt * H + n * 512:mt * H + (n + 1) * 512],
                    in0=acc[:, mt * H + n * 512:mt * H + (n + 1) * 512],
                    in1=pso[:], op=OP.add)

    if DEBUG:
        dma(out=dbg["accs"][:, :], in_=acc[:, 0:H])
    # ---- out: DMA acc -> acc_d; ReduceScatter; store shard ----
    for mt in range(TT):
        dma(out=acc_d[mt * P:(mt + 1) * P, :], in_=acc[:, mt * H:(mt + 1) * H])
    cc = nc.gpsimd.collective_compute(
        "ReduceScatter", OP.add,
        replica_groups=[list(range(NCORES))],
        ins=[acc_d[:, :]], outs=[rs_d[:, :]])
    ccs.append(cc)
    dma(out=out_d[:, :], in_=rs_d[:, :])
    ctx.close()


# ---------------- host side ----------------
_CACHED = {}


def _get_program():
    if "nc" not in _CACHED:
        _CACHED["nc"] = build_program()
    return _CACHED["nc"]


def make_in_maps(hidden_states, w_gate, w1, w2, ws1, ws2):
    x = np.ascontiguousarray(hidden_states, dtype=np.float32)
    xT = np.ascontiguousarray(x.T)
    wgT = np.ascontiguousarray(np.asarray(w_gate, np.float32).T)
    bf = ml_dtypes.bfloat16
    w1 = np.asarray(w1, np.float32)
    w2 = np.asarray(w2, np.float32)
    ws1 = np.asarray(ws1, np.float32)
    ws2 = np.asarray(ws2, np.float32)
    shard = FS // NCORES  # 352
    in_maps = []
    for k in range(NCORES):
        ws1l = np.zeros((H, 2 * SSH), np.float32)
        ws1l[:, :shard] = ws1[:, k * shard:(k + 1) * shard]
        ws1l[:, SSH:SSH + shard] = ws1[:, FS + k * shard:FS + (k + 1) * shard]
        ws2l = np.zeros((SSH, H), np.float32)
        ws2l[:shard] = ws2[k * shard:(k + 1) * shard]
        sel = np.zeros((E, EL), np.float32)
        for e in range(EL):
            sel[k * EL + e, e] = 1.0
        in_maps.append({
            "sel": sel,
            "x": x,
            "xT": xT,
            "wgT": wgT,
            "w1l": np.ascontiguousarray(w1[k * EL:(k + 1) * EL]).astype(bf),
            "w2l": np.ascontiguousarray(w2[k * EL:(k + 1) * EL]).astype(bf),
            "ws1l": ws1l.astype(bf),
            "ws2l": ws2l.astype(bf),
        })
    return in_maps


def kernel(hidden_states, w_gate, w1, w2, ws1, ws2):
    from concourse.bass_utils import run_bass_kernel_spmd
    nc = _get_program()
    in_maps = make_in_maps(hidden_states, w_gate, w1, w2, ws1, ws2)
    res = run_bass_kernel_spmd(nc, in_maps, list(range(NCORES)))
    shards = [res.results[k]["out"] for k in range(NCORES)]
    return np.concatenate(shards, axis=0).astype(np.float32)



# revision 12
# speedup vs baseline: 2.9004x; 2.9004x over previous
"""DeepseekV2 MoE layer on 8 TRN2 NeuronCores (expert-parallel).

Sharding: w1/w2 sharded 4-experts-per-core; router + token activations
replicated; shared expert tensor-parallel along the FS dim (352/core,
zero-padded to 384). Routing (softmax + grouped top-k) computed on device.

Per core: router logits (f32r matmul) -> grouped top-k -> per-expert slot
assignment via triangular-matmul cumsum -> indirect-DMA token gather ->
capacity-C expert MLP in bf16 -> one-hot combine matmul (weights folded
into y) accumulated with the shared-expert down-proj in PSUM -> bf16
ReduceScatter in two h-halves -> each core emits its 128 output rows.

Expert groups are host-permuted per core so the core's own 4 experts are
always rows 0-3 of the routing tensors (group-swap is transparent to the
grouped top-k). All weights are host-pre-tiled into the exact SBUF layout
so each weight load is a single large-descriptor DMA on the sync HWDGE
queue.
"""

import numpy as np
import ml_dtypes

import concourse.bass as bass
import concourse.tile as tile
from concourse import bacc, mybir
from concourse.masks import make_identity

# problem shape
T, H = 1024, 2048
E, F = 32, 1408
F2 = 2 * F                      # 2816
G_GRP, TOPK_G, TOPK = 8, 3, 6
FS = 2 * F                      # 2816 shared intermediate
SCALE = 16.0
NCORES = 8
EL = E // NCORES                # 4 experts per core
C = 256                         # per-expert token capacity (max seen ~212)
P = 128
TT = T // P                     # 8 token tiles
HC = H // P                     # 16 h chunks
FT = F // P                     # 11 f tiles
SSH = 384                       # padded shared shard (352 real)
TRIW = 1920                     # cumsum window width

F32 = mybir.dt.float32
F32R = mybir.dt.float32r
BF16 = mybir.dt.bfloat16
F16 = mybir.dt.float16
I32 = mybir.dt.int32
AF = mybir.ActivationFunctionType
OP = mybir.AluOpType


def build_program():
    nc = bacc.Bacc("TRN2", target_bir_lowering=False, debug=False,
                   num_devices=NCORES)

    xT_d = nc.dram_tensor("xT", [H, T], F32, kind="ExternalInput")
    xbfT_d = nc.dram_tensor("xbfT", [P, HC * T], BF16, kind="ExternalInput")
    xbf_d = nc.dram_tensor("xbf", [T, H], BF16, kind="ExternalInput")
    wg_d = nc.dram_tensor("wgt", [P, HC * E], F32, kind="ExternalInput")
    w1_d = nc.dram_tensor("w1l", [EL, FT, P, HC * 2 * P], BF16,
                          kind="ExternalInput")
    w2_d = nc.dram_tensor("w2l", [EL, 4, P, FT * 512], BF16,
                          kind="ExternalInput")
    ws1_d = nc.dram_tensor("ws1l", [P, HC * 768], BF16, kind="ExternalInput")
    ws2_d = nc.dram_tensor("ws2l", [P, 3 * H], BF16, kind="ExternalInput")
    out_d = nc.dram_tensor("out", [P, H], BF16, kind="ExternalOutput")

    acc_d = [nc.dram_tensor(f"acc{i}", [T, H // 2], BF16) for i in range(2)]
    rs_d = [nc.dram_tensor(f"rs{i}", [P, H // 2], BF16) for i in range(2)]

    with tile.TileContext(nc) as tc:
        _build(nc, tc, xT_d, xbfT_d, xbf_d, wg_d, w1_d, w2_d, ws1_d, ws2_d,
               out_d, acc_d, rs_d)

    nc.compile()
    return nc


def _build(nc, tc, xT_d, xbfT_d, xbf_d, wg_d, w1_d, w2_d, ws1_d, ws2_d,
           out_d, acc_d, rs_d):
    import contextlib
    ctx = contextlib.ExitStack()
    sbA = ctx.enter_context(tc.tile_pool(name="sbA", bufs=1))
    ps = ctx.enter_context(tc.tile_pool(name="ps", bufs=1, space="PSUM"))

    def pA(name):
        return ps.tile([P, 512], F32, tag="pA", bufs=4, name=name)

    def pB(name, dt=F32):
        return ps.tile([P, C], dt, tag="pB", bufs=4, name=name)

    # ---- constants ----
    ident = sbA.tile([P, P], F32)
    make_identity(nc, ident[:])
    identb = sbA.tile([P, P], BF16)
    make_identity(nc, identb[:])
    iota_ci = sbA.tile([P, C], I32)
    nc.gpsimd.iota(iota_ci[:], pattern=[[1, C]], base=0, channel_multiplier=0)
    iota_c = sbA.tile([P, C], F32)
    nc.vector.tensor_copy(iota_c[:], iota_ci[:])
    ihalf_i = sbA.tile([P, 2], I32)      # col h: value 128*h + p
    nc.gpsimd.iota(ihalf_i[:], pattern=[[P, 2]], base=0, channel_multiplier=1)
    ihalf_bf = sbA.tile([P, 2], BF16)
    nc.vector.tensor_copy(ihalf_bf[:], ihalf_i[:])
    toki = sbA.tile([P, TT], I32)        # col k: value 128*k + p
    nc.gpsimd.iota(toki[:], pattern=[[P, TT]], base=0, channel_multiplier=1)
    tok5 = sbA.tile([P, TT * 5], F32R)   # per k: [tok | w_e0..w_e3]
    tok5v = tok5[:].rearrange("p (k c) -> p k c", c=5)
    nc.vector.tensor_copy(tok5v[:, :, 0:1], toki[:].rearrange(
        "p (k c) -> p k c", c=1))
    tri = sbA.tile([P, TRIW], BF16)      # tri[p, q] = (q >= p + 897)
    nc.vector.memset(tri[:], 1.0)
    nc.gpsimd.affine_select(out=tri[:], in_=tri[:], pattern=[[1, TRIW]],
                            compare_op=OP.is_ge, fill=0.0, base=-897,
                            channel_multiplier=-1)
    sel4 = sbA.tile([P, EL * P], BF16)   # [k, e*128+m] = (k == e)
    nc.vector.memset(sel4[:], 1.0)
    nc.gpsimd.affine_select(out=sel4[:], in_=sel4[:], pattern=[[1, EL * P]],
                            compare_op=OP.is_ge, fill=0.0, base=0,
                            channel_multiplier=-P)
    nc.gpsimd.affine_select(out=sel4[:], in_=sel4[:], pattern=[[-1, EL * P]],
                            compare_op=OP.is_ge, fill=0.0, base=P - 1,
                            channel_multiplier=P)

    # ---- persistent tiles ----
    act_sT = sbA.tile([P, 3 * T], BF16)          # shared act^T [fs, t]
    slotT_bf = sbA.tile([E, T], BF16)
    posTok = sbA.tile([P, TT * EL], F32)         # slot of token, local experts
    ws2_sb = sbA.tile([P, 3 * H], BF16)
    y_sb = [sbA.tile([P, 2 * H], BF16, tag=f"y{e}", name=f"y{e}")
            for e in range(EL)]
    gmat = [sbA.tile([P, 2 * T], BF16, tag=f"g{e}", name=f"gm{e}")
            for e in range(EL)]
    wslot = [sbA.tile([P, 2], F32, tag=f"w{e}", name=f"ws{e}")
             for e in range(EL)]

    nc.sync.dma_start(out=ws2_sb[:], in_=ws2_d[:, :])

    # ================= phase 1: router + shared up-proj + routing =========
    with tc.tile_pool(name="sbB", bufs=1) as sbB:
        wg_sb = sbB.tile([P, HC * E], F32)
        nc.sync.dma_start(out=wg_sb[:], in_=wg_d[:, :])
        wgr = sbB.tile([P, HC * E], F32R)
        nc.vector.tensor_copy(wgr[:], wg_sb[:])
        xbfT = sbB.tile([P, HC * T], BF16)
        for q in range(4):
            w = HC * T // 4
            nc.sync.dma_start(out=xbfT[:, q * w:(q + 1) * w],
                              in_=xbfT_d[:, q * w:(q + 1) * w])
        ws1_sb = sbB.tile([P, HC * 768], BF16)
        nc.sync.dma_start(out=ws1_sb[:], in_=ws1_d[:, :])

        logT = sbB.tile([E, T], F32)
        # router: logitsT = wgT.T @ x.T ; f32r single-pass
        psl = [ps.tile([E, T // 2], F32, tag="pA", bufs=4, name=f"psl{n}")
               for n in range(2)]
        for k in range(HC):
            xt = sbB.tile([P, T], F32, tag="xt", bufs=2, name=f"xt{k}")
            nc.sync.dma_start(out=xt[:], in_=xT_d[k * P:(k + 1) * P, :])
            xtr = sbB.tile([P, T], F32R, tag="xtr", bufs=4, name=f"xtr{k}")
            nc.vector.tensor_copy(xtr[:], xt[:])
            for n in range(2):
                nc.tensor.matmul(
                    psl[n][:], wgr[:, k * E:(k + 1) * E],
                    xtr[:, n * (T // 2):(n + 1) * (T // 2)],
                    start=(k == 0), stop=(k == HC - 1))
        for n in range(2):
            nc.vector.tensor_copy(logT[:, n * (T // 2):(n + 1) * (T // 2)],
                                  psl[n][:])

        # transpose logitsT -> scores [128, (k e)]
        scores = sbB.tile([P, TT * E], F32)
        for k in range(TT):
            pst = pB(f"ltr{k}")
            nc.tensor.transpose(pst[:, :E], logT[:, k * P:(k + 1) * P],
                                ident[:E, :E])
            nc.vector.tensor_copy(scores[:, k * E:(k + 1) * E], pst[:, :E])

        # shared expert up-proj: act_sT[fs, t] (3 fs-tiles x 2 t-halves)
        for mg in range(3):
            for n in range(2):
                psg = pA(f"psg{mg}{n}")
                psu = pA(f"psu{mg}{n}")
                for k in range(HC):
                    rhs = xbfT[:, k * T + n * 512:k * T + (n + 1) * 512]
                    nc.tensor.matmul(psg[:],
                                     ws1_sb[:, k * 768 + mg * P:
                                            k * 768 + (mg + 1) * P],
                                     rhs, start=(k == 0), stop=(k == HC - 1))
                    nc.tensor.matmul(psu[:],
                                     ws1_sb[:, k * 768 + SSH + mg * P:
                                            k * 768 + SSH + (mg + 1) * P],
                                     rhs, start=(k == 0), stop=(k == HC - 1))
                gsil = sbB.tile([P, 512], F32, tag="gsil", bufs=2,
                                name=f"gsil{mg}{n}")
                nc.scalar.activation(gsil[:], psg[:], AF.Sigmoid)
                nc.vector.tensor_tensor(out=gsil[:], in0=gsil[:], in1=psg[:],
                                        op=OP.mult)
                nc.vector.tensor_tensor(
                    out=act_sT[:, mg * T + n * 512:mg * T + (n + 1) * 512],
                    in0=gsil[:], in1=psu[:], op=OP.mult)

        # softmax + grouped top-k (per t-tile)
        comb = sbB.tile([P, TT * E], F32)
        mask_bf = sbB.tile([P, TT * E], BF16)
        tmp8 = sbB.tile([P, 8], F32)
        for k in range(TT):
            blk = scores[:, k * E:(k + 1) * E]
            mx = sbB.tile([P, 1], F32, tag="rmax", bufs=2, name=f"rmax{k}")
            nc.vector.tensor_reduce(mx[:], blk, axis=mybir.AxisListType.X,
                                    op=OP.max, negate=True)
            sm = sbB.tile([P, 1], F32, tag="rsum", bufs=2, name=f"rsum{k}")
            nc.scalar.activation(blk, blk, AF.Exp, bias=mx[:], accum_out=sm[:])
            rc = sbB.tile([P, 1], F32, tag="rrec", bufs=2, name=f"rrec{k}")
            nc.vector.reciprocal(rc[:], sm[:])
            nc.vector.tensor_scalar_mul(blk, blk, rc[:])

            blk3 = blk.rearrange("p (g f) -> p g f", f=4)
            gsc = sbB.tile([P, G_GRP], F32, tag="gsc", bufs=2, name=f"gsc{k}")
            nc.vector.tensor_reduce(gsc[:], blk3, axis=mybir.AxisListType.X,
                                    op=OP.max)
            nc.vector.max(out=tmp8[:], in_=gsc[:])
            nc.vector.memset(tmp8[:, TOPK_G:], 0.0)
            gz = sbB.tile([P, G_GRP], F32, tag="gz", bufs=2, name=f"gz{k}")
            nc.vector.match_replace(out=gz[:], in_to_replace=tmp8[:],
                                    in_values=gsc[:], imm_value=0.0)
            nc.vector.tensor_tensor(out=gz[:], in0=gsc[:], in1=gz[:],
                                    op=OP.subtract)
            nc.vector.tensor_scalar(gz[:], gz[:], 0.0, scalar2=None,
                                    op0=OP.is_gt)
            cblk = comb[:, k * E:(k + 1) * E]
            cblk3 = cblk.rearrange("p (g f) -> p g f", f=4)
            gz3 = gz[:].rearrange("p (g o) -> p g o", o=1)
            nc.vector.tensor_tensor(out=cblk3, in0=blk3,
                                    in1=gz3.to_broadcast([P, G_GRP, 4]),
                                    op=OP.mult)
            nc.vector.max(out=tmp8[:], in_=cblk)
            nc.vector.memset(tmp8[:, TOPK:], 0.0)
            zap = sbB.tile([P, E], F32, tag="zap", bufs=2, name=f"zap{k}")
            nc.vector.match_replace(out=zap[:], in_to_replace=tmp8[:],
                                    in_values=cblk, imm_value=0.0)
            nc.vector.tensor_tensor(out=cblk, in0=cblk, in1=zap[:],
                                    op=OP.subtract)
            nc.vector.tensor_scalar_mul(cblk, cblk, SCALE)
            nc.vector.tensor_scalar(mask_bf[:, k * E:(k + 1) * E], cblk, 0.0,
                                    scalar2=None, op0=OP.is_gt)
        # local-expert weights into the slot-matmul lhsT (cols 1..4 per k)
        nc.vector.tensor_copy(
            tok5v[:, :, 1:5],
            comb[:].rearrange("p (k e) -> p k e", e=E)[:, :, 0:EL])

        # strict cumsum: pos[e, t] = sum_{t' < t} mask[e, t']
        pos = sbB.tile([E, T], F32)
        for n in range(2):
            psc = ps.tile([E, T // 2], F32, tag="pA", bufs=4, name=f"psc{n}")
            for k in range(TT):
                s = 896 - k * P + n * 512
                nc.tensor.matmul(psc[:], mask_bf[:, k * E:(k + 1) * E],
                                 tri[:, s:s + 512],
                                 start=(k == 0), stop=(k == TT - 1))
            nc.vector.tensor_copy(pos[:, n * (T // 2):(n + 1) * (T // 2)],
                                  psc[:])
        # maskT via PE transpose of mask_bf
        maskT = sbB.tile([E, T], F32)
        for k in range(TT):
            ptm = pB(f"mtr{k}", BF16)
            nc.tensor.transpose(ptm[:E, :P], mask_bf[:, k * E:(k + 1) * E],
                                identb[:])
            nc.vector.tensor_copy(maskT[:, k * P:(k + 1) * P], ptm[:E, :P])
        # slot[e,t] = mask ? pos : C   == C + (pos - C) * mask
        nc.vector.tensor_scalar(pos[:], pos[:], float(C), scalar2=None,
                                op0=OP.subtract)
        nc.vector.tensor_tensor(out=pos[:], in0=pos[:], in1=maskT[:],
                                op=OP.mult)
        nc.vector.tensor_scalar(pos[:], pos[:], float(C), scalar2=None,
                                op0=OP.add)
        nc.vector.tensor_copy(slotT_bf[:], pos[:])
        # posTok: slot values token-major for the local experts
        for k in range(TT):
            ptt = pB(f"ptr{k}")
            nc.tensor.transpose(ptt[:, :E], pos[:, k * P:(k + 1) * P],
                                ident[:E, :E])
            nc.vector.tensor_copy(posTok[:, k * EL:(k + 1) * EL],
                                  ptt[:, 0:EL])

    # ================= phase 2: expert prep + MLPs ========================
    with tc.tile_pool(name="sbC", bufs=1) as sbC:
        xet = [sbC.tile([P, HC * C], BF16, tag=f"xet{e}", name=f"xet{e}")
               for e in range(EL)]
        # ---- prep all experts: srow, slot->token, gather, transpose ----
        for e in range(EL):
            # srow: broadcast slotT row e to all partitions (K=4 matmul)
            srow = sbC.tile([P, T], BF16, tag="srow", bufs=2, name=f"srow{e}")
            for n in range(2):
                psb = pA(f"srow{e}{n}")
                nc.tensor.matmul(psb[:], sel4[0:E, e * P:(e + 1) * P],
                                 slotT_bf[0:E, n * 512:(n + 1) * 512],
                                 start=True, stop=True)
                nc.vector.tensor_copy(srow[:, n * 512:(n + 1) * 512], psb[:])
            for half in range(2):
                nc.vector.tensor_tensor(
                    out=gmat[e][:, half * T:(half + 1) * T],
                    in0=ihalf_bf[:, half:half + 1].to_broadcast([P, T]),
                    in1=srow[:], op=OP.is_equal)
            # slot -> (token id, weight): accumulate over t-tiles
            pstk = ps.tile([5, C], F32, tag="pB", bufs=4, name=f"stk{e}")
            for k in range(TT):
                ptk = sbC.tile([P, C], F32R, tag="petk", bufs=2,
                               name=f"petk{e}{k}")
                nc.vector.tensor_tensor(
                    out=ptk[:],
                    in0=posTok[:, k * EL + e:k * EL + e + 1].to_broadcast(
                        [P, C]),
                    in1=iota_c[:], op=OP.is_equal)
                nc.tensor.matmul(pstk[:], tok5[:, k * 5:(k + 1) * 5], ptk[:],
                                 start=(k == 0), stop=(k == TT - 1))
            stokw = sbC.tile([E, C], F32, tag="stokw", bufs=2,
                             name=f"stokw{e}")
            nc.vector.memset(stokw[:], 0.0)
            nc.vector.tensor_copy(stokw[0:5, :], pstk[:])
            stok_i = sbC.tile([P, 2], I32, tag="stoki", bufs=2,
                              name=f"stoki{e}")
            for half in range(2):
                ptt = pB(f"st{e}{half}")
                nc.tensor.transpose(ptt[:, 0:E],
                                    stokw[0:E, half * P:(half + 1) * P],
                                    ident[0:E, 0:E])
                nc.vector.tensor_copy(stok_i[:, half:half + 1], ptt[:, 0:1])
                nc.vector.tensor_copy(wslot[e][:, half:half + 1],
                                      ptt[:, 1 + e:2 + e])
            # gather x rows and transpose to [h, c]
            for half in range(2):
                xe = sbC.tile([P, H], BF16, tag="xe", bufs=2,
                              name=f"xe{e}{half}")
                nc.gpsimd.indirect_dma_start(
                    out=xe[:], out_offset=None, in_=xbf_d[:, :],
                    in_offset=bass.IndirectOffsetOnAxis(
                        ap=stok_i[:, half:half + 1], axis=0))
                for hc in range(HC):
                    ptx = pB(f"xt{e}{half}{hc}", BF16)
                    nc.tensor.transpose(ptx[:, :P],
                                        xe[:, hc * P:(hc + 1) * P],
                                        identb[:])
                    nc.vector.tensor_copy(
                        xet[e][:, hc * C + half * P:hc * C + half * P + P],
                        ptx[:, :P])

        # ---- MM1 + MM2 per expert ----
        for e in range(EL):
            act_e = sbC.tile([P, FT * C], BF16, tag="act", bufs=2,
                             name=f"act{e}")
            for m in range(FT):
                w1p = sbC.tile([P, HC * 2 * P], BF16, tag="w1p", bufs=3,
                               name=f"w1p{e}{m}")
                nc.sync.dma_start(out=w1p[:], in_=w1_d[e, m, :, :])
                psg = pB(f"mg{e}{m}")
                psu = pB(f"mu{e}{m}")
                for k in range(HC):
                    rhs = xet[e][:, k * C:(k + 1) * C]
                    nc.tensor.matmul(psg[:], w1p[:, k * 2 * P:k * 2 * P + P],
                                     rhs, start=(k == 0), stop=(k == HC - 1))
                    nc.tensor.matmul(psu[:],
                                     w1p[:, k * 2 * P + P:(k + 1) * 2 * P],
                                     rhs, start=(k == 0), stop=(k == HC - 1))
                sgt = sbC.tile([P, C], F32, tag="sgt", bufs=2,
                               name=f"sgt{e}{m}")
                nc.scalar.activation(sgt[:], psg[:], AF.Sigmoid)
                nc.vector.tensor_tensor(out=sgt[:], in0=sgt[:], in1=psg[:],
                                        op=OP.mult)
                nc.vector.tensor_tensor(out=act_e[:, m * C:(m + 1) * C],
                                        in0=sgt[:], in1=psu[:], op=OP.mult)
            for n in range(4):
                w2t = sbC.tile([P, FT * 512], BF16, tag="w2t", bufs=2,
                               name=f"w2t{e}{n}")
                nc.sync.dma_start(out=w2t[:], in_=w2_d[e, n, :, :])
                psy = [pA(f"y{e}{n}{mc}") for mc in range(2)]
                for kf in range(FT):
                    for mc in range(2):
                        nc.tensor.matmul(
                            psy[mc][:],
                            act_e[:, kf * C + mc * P:kf * C + (mc + 1) * P],
                            w2t[:, kf * 512:(kf + 1) * 512],
                            start=(kf == 0), stop=(kf == FT - 1))
                for mc in range(2):
                    nc.vector.tensor_scalar_mul(
                        y_sb[e][:, mc * H + n * 512:mc * H + (n + 1) * 512],
                        psy[mc][:], wslot[e][:, mc:mc + 1])

        # ================= phase 3: combine + ReduceScatter ===============
        for hh in range(2):
            for mt in range(TT):
                for nn in range(2):
                    pc = pA(f"c{hh}{mt}{nn}")
                    col = hh * 1024 + nn * 512
                    for kf in range(3):
                        nc.tensor.matmul(
                            pc[:],
                            act_sT[:, kf * T + mt * P:kf * T + (mt + 1) * P],
                            ws2_sb[:, kf * H + col:kf * H + col + 512],
                            start=(kf == 0), stop=False)
                    cnt = 3
                    for e in range(EL):
                        for ch in range(2):
                            cnt += 1
                            nc.tensor.matmul(
                                pc[:],
                                gmat[e][:, ch * T + mt * P:
                                        ch * T + (mt + 1) * P],
                                y_sb[e][:, ch * H + col:ch * H + col + 512],
                                start=False, stop=(cnt == 11))
                    ob = sbC.tile([P, 512], BF16, tag="ob", bufs=4,
                                  name=f"ob{hh}{mt}{nn}")
                    nc.vector.tensor_copy(ob[:], pc[:])
                    nc.sync.dma_start(
                        out=acc_d[hh][mt * P:(mt + 1) * P,
                                      nn * 512:(nn + 1) * 512],
                        in_=ob[:])
            nc.gpsimd.collective_compute(
                "ReduceScatter", OP.add,
                replica_groups=[list(range(NCORES))],
                ins=[acc_d[hh][:, :]], outs=[rs_d[hh][:, :]])
            nc.sync.dma_start(out=out_d[:, hh * 1024:(hh + 1) * 1024],
                              in_=rs_d[hh][:, :])

    ctx.close()


# ---------------- host side ----------------
_CACHED = {}


def _get_program():
    if "nc" not in _CACHED:
        _CACHED["nc"] = build_program()
    return _CACHED["nc"]


def make_in_maps(hidden_states, w_gate, w1, w2, ws1, ws2):
    bf = ml_dtypes.bfloat16
    x = np.ascontiguousarray(hidden_states, dtype=np.float32)
    xT = np.ascontiguousarray(x.T)                                 # [H, T]
    xbfT = np.ascontiguousarray(
        xT.reshape(HC, P, T).transpose(1, 0, 2).reshape(P, HC * T)).astype(bf)
    xbf = x.astype(bf)                                             # [T, H]
    wg = np.asarray(w_gate, np.float32)                            # [E, H]
    w1 = np.asarray(w1, np.float32)
    w2 = np.asarray(w2, np.float32)
    ws1 = np.asarray(ws1, np.float32)
    ws2 = np.asarray(ws2, np.float32)

    # w1 interleaved (gate_m | up_m) then tiled [E, FT, P, HC*256]
    w1c = np.concatenate([w1[:, :, :F].reshape(E, H, FT, P),
                          w1[:, :, F:].reshape(E, H, FT, P)], axis=3)
    w1t = np.ascontiguousarray(
        w1c.reshape(E, HC, P, FT, 2 * P).transpose(0, 3, 2, 1, 4).reshape(
            E, FT, P, HC * 2 * P)).astype(bf)
    # w2 tiled [E, 4, P, FT*512]
    w2t = np.ascontiguousarray(
        w2.reshape(E, FT, P, 4, 512).transpose(0, 3, 2, 1, 4).reshape(
            E, 4, P, FT * 512)).astype(bf)

    shard = FS // NCORES  # 352
    in_maps = []
    for k in range(NCORES):
        # shared expert shard, padded 352 -> 384, tiled [P, HC*768]
        ws1p = np.zeros((H, 2 * SSH), np.float32)
        ws1p[:, :shard] = ws1[:, k * shard:(k + 1) * shard]
        ws1p[:, SSH:SSH + shard] = ws1[:, FS + k * shard:FS + (k + 1) * shard]
        ws1l = np.ascontiguousarray(
            ws1p.reshape(HC, P, 2 * SSH).transpose(1, 0, 2).reshape(
                P, HC * 768)).astype(bf)
        ws2p = np.zeros((SSH, H), np.float32)
        ws2p[:shard] = ws2[k * shard:(k + 1) * shard]
        ws2l = np.ascontiguousarray(
            ws2p.reshape(3, P, H).transpose(1, 0, 2).reshape(
                P, 3 * H)).astype(bf)
        # group permutation: swap group 0 <-> group k so the core's own
        # 4 experts are rows 0..3 (grouped top-k is group-order invariant)
        gperm = list(range(G_GRP))
        gperm[0], gperm[k] = gperm[k], gperm[0]
        eperm = [g * EL + i for g in gperm for i in range(EL)]
        wgp = wg[eperm]                                            # [E, H]
        wgt = np.ascontiguousarray(
            wgp.T.reshape(HC, P, E).transpose(1, 0, 2).reshape(
                P, HC * E)).astype(np.float32)
        in_maps.append({
            "xT": xT,
            "xbfT": xbfT,
            "xbf": xbf,
            "wgt": wgt,
            "w1l": np.ascontiguousarray(w1t[k * EL:(k + 1) * EL]),
            "w2l": np.ascontiguousarray(w2t[k * EL:(k + 1) * EL]),
            "ws1l": ws1l,
            "ws2l": ws2l,
        })
    return in_maps


def kernel(hidden_states, w_gate, w1, w2, ws1, ws2):
    from concourse.bass_utils import run_bass_kernel_spmd
    nc = _get_program()
    in_maps = make_in_maps(hidden_states, w_gate, w1, w2, ws1, ws2)
    res = run_bass_kernel_spmd(nc, in_maps, list(range(NCORES)))
    shards = [res.results[k]["out"] for k in range(NCORES)]
    return np.concatenate(shards, axis=0).astype(np.float32)


# revision 20
# speedup vs baseline: 2.9049x; 1.0015x over previous
"""DeepseekV2 MoE layer on 8 TRN2 NeuronCores (expert-parallel).

Sharding: w1/w2 sharded 4-experts-per-core; router + token activations
replicated; shared expert tensor-parallel along the FS dim (352/core,
zero-padded to 384). Routing (softmax + grouped top-k) computed on device.

Per core: router logits (f32r matmul) -> grouped top-k -> per-expert slot
assignment via triangular-matmul cumsum -> indirect-DMA token gather ->
capacity-C expert MLP in bf16 -> one-hot combine matmul (weights folded
into y) accumulated with the shared-expert down-proj in PSUM -> bf16
ReduceScatter in two h-halves -> each core emits its 128 output rows.

Expert groups are host-permuted per core so the core's own 4 experts are
always rows 0-3 of the routing tensors (group-swap is transparent to the
grouped top-k). All weights are host-pre-tiled into the exact SBUF layout
so each weight load is a single large-descriptor DMA on the sync HWDGE
queue.
"""

import numpy as np
import ml_dtypes

import concourse.bass as bass
import concourse.tile as tile
from concourse import bacc, mybir
from concourse.masks import make_identity

# problem shape
T, H = 1024, 2048
E, F = 32, 1408
F2 = 2 * F                      # 2816
G_GRP, TOPK_G, TOPK = 8, 3, 6
FS = 2 * F                      # 2816 shared intermediate
SCALE = 16.0
NCORES = 8
EL = E // NCORES                # 4 experts per core
C = 256                         # per-expert token capacity (max seen ~212)
P = 128
TT = T // P                     # 8 token tiles
HC = H // P                     # 16 h chunks
FT = F // P                     # 11 f tiles
SSH = 384                       # padded shared shard (352 real)
TRIW = 1920                     # cumsum window width

F32 = mybir.dt.float32
F32R = mybir.dt.float32r
BF16 = mybir.dt.bfloat16
F16 = mybir.dt.float16
I32 = mybir.dt.int32
AF = mybir.ActivationFunctionType
OP = mybir.AluOpType


def build_program():
    nc = bacc.Bacc("TRN2", target_bir_lowering=False, debug=False,
                   num_devices=NCORES)

    xT_d = nc.dram_tensor("xT", [H, T], F32, kind="ExternalInput")
    xbfT_d = nc.dram_tensor("xbfT", [P, HC * T], BF16, kind="ExternalInput")
    xbf_d = nc.dram_tensor("xbf", [T, H], BF16, kind="ExternalInput")
    wg_d = nc.dram_tensor("wgt", [P, HC * E], F32, kind="ExternalInput")
    w1_d = nc.dram_tensor("w1l", [EL, FT, P, HC * 2 * P], BF16,
                          kind="ExternalInput")
    w2_d = nc.dram_tensor("w2l", [EL, 4, P, FT * 512], BF16,
                          kind="ExternalInput")
    ws1_d = nc.dram_tensor("ws1l", [P, HC * 768], BF16, kind="ExternalInput")
    ws2_d = nc.dram_tensor("ws2l", [P, 3 * H], BF16, kind="ExternalInput")
    out_d = nc.dram_tensor("out", [P, H], BF16, kind="ExternalOutput")

    acc_d = [nc.dram_tensor(f"acc{i}", [T, H // 2], BF16) for i in range(2)]
    rs_d = [nc.dram_tensor(f"rs{i}", [P, H // 2], BF16) for i in range(2)]

    with tile.TileContext(nc) as tc:
        _build(nc, tc, xT_d, xbfT_d, xbf_d, wg_d, w1_d, w2_d, ws1_d, ws2_d,
               out_d, acc_d, rs_d)

    nc.compile()
    return nc


def _build(nc, tc, xT_d, xbfT_d, xbf_d, wg_d, w1_d, w2_d, ws1_d, ws2_d,
           out_d, acc_d, rs_d):
    import contextlib
    ctx = contextlib.ExitStack()
    sbA = ctx.enter_context(tc.tile_pool(name="sbA", bufs=1))
    ps = ctx.enter_context(tc.tile_pool(name="ps", bufs=1, space="PSUM"))

    def pA(name):
        return ps.tile([P, 512], F32, tag="pA", bufs=4, name=name)

    def pB(name, dt=F32):
        return ps.tile([P, C], dt, tag="pB", bufs=4, name=name)

    # ---- constants ----
    ident = sbA.tile([P, P], F32)
    make_identity(nc, ident[:])
    identb = sbA.tile([P, P], BF16)
    make_identity(nc, identb[:])
    iota_ci = sbA.tile([P, C], I32)
    nc.gpsimd.iota(iota_ci[:], pattern=[[1, C]], base=0, channel_multiplier=0)
    iota_c = sbA.tile([P, C], F32)
    nc.vector.tensor_copy(iota_c[:], iota_ci[:])
    ihalf_i = sbA.tile([P, 2], I32)      # col h: value 128*h + p
    nc.gpsimd.iota(ihalf_i[:], pattern=[[P, 2]], base=0, channel_multiplier=1)
    ihalf_bf = sbA.tile([P, 2], BF16)
    nc.vector.tensor_copy(ihalf_bf[:], ihalf_i[:])
    toki = sbA.tile([P, TT], I32)        # col k: value 128*k + p
    nc.gpsimd.iota(toki[:], pattern=[[P, TT]], base=0, channel_multiplier=1)
    tok5 = sbA.tile([P, TT * 5], F32R)   # per k: [tok | w_e0..w_e3]
    tok5v = tok5[:].rearrange("p (k c) -> p k c", c=5)
    nc.vector.tensor_copy(tok5v[:, :, 0:1], toki[:].rearrange(
        "p (k c) -> p k c", c=1))
    tri = sbA.tile([P, TRIW], BF16)      # tri[p, q] = (q >= p + 897)
    nc.vector.memset(tri[:], 1.0)
    nc.gpsimd.affine_select(out=tri[:], in_=tri[:], pattern=[[1, TRIW]],
                            compare_op=OP.is_ge, fill=0.0, base=-897,
                            channel_multiplier=-1)
    sel4 = sbA.tile([P, EL * P], BF16)   # [k, e*128+m] = (k == e)
    nc.vector.memset(sel4[:], 1.0)
    nc.gpsimd.affine_select(out=sel4[:], in_=sel4[:], pattern=[[1, EL * P]],
                            compare_op=OP.is_ge, fill=0.0, base=0,
                            channel_multiplier=-P)
    nc.gpsimd.affine_select(out=sel4[:], in_=sel4[:], pattern=[[-1, EL * P]],
                            compare_op=OP.is_ge, fill=0.0, base=P - 1,
                            channel_multiplier=P)

    # ---- persistent tiles ----
    act_sT = sbA.tile([P, 3 * T], BF16)          # shared act^T [fs, t]
    slotT_bf = sbA.tile([E, T], BF16)
    posTok = sbA.tile([P, TT * EL], F32)         # slot of token, local experts
    y_sb = [sbA.tile([P, 2 * H], BF16, tag=f"y{e}", name=f"y{e}")
            for e in range(EL)]
    gmat = [sbA.tile([P, 2 * T], BF16, tag=f"g{e}", name=f"gm{e}")
            for e in range(EL)]
    wslot = [sbA.tile([P, 2], F32, tag=f"w{e}", name=f"ws{e}")
             for e in range(EL)]
    xet = [sbA.tile([P, HC * C], BF16, tag=f"xet{e}", name=f"xet{e}")
           for e in range(EL)]

    # ================= phase 1: router + shared up-proj + routing =========
    with tc.tile_pool(name="sbB", bufs=1) as sbB:
        wg_sb = sbB.tile([P, HC * E], F32)
        nc.sync.dma_start(out=wg_sb[:], in_=wg_d[:, :])
        wgr = sbB.tile([P, HC * E], F32R)
        nc.vector.tensor_copy(wgr[:], wg_sb[:])

        logT = sbB.tile([E, T], F32)
        # router: logitsT = wgT.T @ x.T ; f32r single-pass
        psl = [ps.tile([E, T // 2], F32, tag="pA", bufs=4, name=f"psl{n}")
               for n in range(2)]
        for k in range(HC):
            xt = sbB.tile([P, T], F32, tag="xt", bufs=2, name=f"xt{k}")
            nc.sync.dma_start(out=xt[:], in_=xT_d[k * P:(k + 1) * P, :])
            xtr = sbB.tile([P, T], F32R, tag="xtr", bufs=2, name=f"xtr{k}")
            nc.vector.tensor_copy(xtr[:], xt[:])
            for n in range(2):
                nc.tensor.matmul(
                    psl[n][:], wgr[:, k * E:(k + 1) * E],
                    xtr[:, n * (T // 2):(n + 1) * (T // 2)],
                    start=(k == 0), stop=(k == HC - 1))
        for n in range(2):
            nc.vector.tensor_copy(logT[:, n * (T // 2):(n + 1) * (T // 2)],
                                  psl[n][:])

        # bulk loads on the scalar HWDGE queue (parallel to xt stream)
        xbfT = sbB.tile([P, HC * T], BF16)
        for q in range(4):
            w = HC * T // 4
            nc.scalar.dma_start(out=xbfT[:, q * w:(q + 1) * w],
                                in_=xbfT_d[:, q * w:(q + 1) * w])
        ws1_sb = sbB.tile([P, HC * 768], BF16)
        nc.scalar.dma_start(out=ws1_sb[:], in_=ws1_d[:, :])

        # transpose logitsT -> scores [128, (k e)]
        scores = sbB.tile([P, TT * E], F32)
        for k in range(TT):
            pst = pB(f"ltr{k}")
            nc.tensor.transpose(pst[:, :E], logT[:, k * P:(k + 1) * P],
                                ident[:E, :E])
            nc.vector.tensor_copy(scores[:, k * E:(k + 1) * E], pst[:, :E])

        # shared expert up-proj: act_sT[fs, t] (3 fs-tiles x 2 t-halves),
        # emitted in blocks interleaved with routing tail + expert prep so
        # the PE stays busy while routing/gather dependencies resolve
        def s1_block(mg):
            for n in range(2):
                psg = pA(f"psg{mg}{n}")
                psu = pA(f"psu{mg}{n}")
                for k in range(HC):
                    rhs = xbfT[:, k * T + n * 512:k * T + (n + 1) * 512]
                    nc.tensor.matmul(psg[:],
                                     ws1_sb[:, k * 768 + mg * P:
                                            k * 768 + (mg + 1) * P],
                                     rhs, start=(k == 0), stop=(k == HC - 1))
                    nc.tensor.matmul(psu[:],
                                     ws1_sb[:, k * 768 + SSH + mg * P:
                                            k * 768 + SSH + (mg + 1) * P],
                                     rhs, start=(k == 0), stop=(k == HC - 1))
                gsil = sbB.tile([P, 512], F32, tag="gsil", bufs=2,
                                name=f"gsil{mg}{n}")
                nc.scalar.activation(gsil[:], psg[:], AF.Sigmoid)
                nc.vector.tensor_tensor(out=gsil[:], in0=gsil[:], in1=psg[:],
                                        op=OP.mult)
                nc.vector.tensor_tensor(
                    out=act_sT[:, mg * T + n * 512:mg * T + (n + 1) * 512],
                    in0=gsil[:], in1=psu[:], op=OP.mult)

        s1_block(0)

        # softmax + grouped top-k (per t-tile)
        comb = sbB.tile([P, TT * E], F32)
        mask_bf = sbB.tile([P, TT * E], BF16)
        tmp8 = sbB.tile([P, 8], F32)
        for k in range(TT):
            blk = scores[:, k * E:(k + 1) * E]
            mx = sbB.tile([P, 1], F32, tag="rmax", bufs=2, name=f"rmax{k}")
            nc.vector.tensor_reduce(mx[:], blk, axis=mybir.AxisListType.X,
                                    op=OP.max, negate=True)
            sm = sbB.tile([P, 1], F32, tag="rsum", bufs=2, name=f"rsum{k}")
            nc.scalar.activation(blk, blk, AF.Exp, bias=mx[:], accum_out=sm[:])
            rc = sbB.tile([P, 1], F32, tag="rrec", bufs=2, name=f"rrec{k}")
            nc.vector.reciprocal(rc[:], sm[:])
            nc.vector.tensor_scalar_mul(blk, blk, rc[:])

            blk3 = blk.rearrange("p (g f) -> p g f", f=4)
            gsc = sbB.tile([P, G_GRP], F32, tag="gsc", bufs=2, name=f"gsc{k}")
            nc.vector.tensor_reduce(gsc[:], blk3, axis=mybir.AxisListType.X,
                                    op=OP.max)
            nc.vector.max(out=tmp8[:], in_=gsc[:])
            nc.vector.memset(tmp8[:, TOPK_G:], 0.0)
            gz = sbB.tile([P, G_GRP], F32, tag="gz", bufs=2, name=f"gz{k}")
            nc.vector.match_replace(out=gz[:], in_to_replace=tmp8[:],
                                    in_values=gsc[:], imm_value=0.0)
            nc.vector.tensor_tensor(out=gz[:], in0=gsc[:], in1=gz[:],
                                    op=OP.subtract)
            nc.vector.tensor_scalar(gz[:], gz[:], 0.0, scalar2=None,
                                    op0=OP.is_gt)
            cblk = comb[:, k * E:(k + 1) * E]
            cblk3 = cblk.rearrange("p (g f) -> p g f", f=4)
            gz3 = gz[:].rearrange("p (g o) -> p g o", o=1)
            nc.vector.tensor_tensor(out=cblk3, in0=blk3,
                                    in1=gz3.to_broadcast([P, G_GRP, 4]),
                                    op=OP.mult)
            nc.vector.max(out=tmp8[:], in_=cblk)
            nc.vector.memset(tmp8[:, TOPK:], 0.0)
            zap = sbB.tile([P, E], F32, tag="zap", bufs=2, name=f"zap{k}")
            nc.vector.match_replace(out=zap[:], in_to_replace=tmp8[:],
                                    in_values=cblk, imm_value=0.0)
            nc.vector.tensor_tensor(out=cblk, in0=cblk, in1=zap[:],
                                    op=OP.subtract)
            nc.vector.tensor_scalar_mul(cblk, cblk, SCALE)
            nc.vector.tensor_scalar(mask_bf[:, k * E:(k + 1) * E], cblk, 0.0,
                                    scalar2=None, op0=OP.is_gt)
        # local-expert weights into the slot-matmul lhsT (cols 1..4 per k)
        nc.vector.tensor_copy(
            tok5v[:, :, 1:5],
            comb[:].rearrange("p (k e) -> p k e", e=E)[:, :, 0:EL])

        # strict cumsum: pos[e, t] = sum_{t' < t} mask[e, t']
        pos = sbB.tile([E, T], F32)
        for n in range(2):
            psc = ps.tile([E, T // 2], F32, tag="pA", bufs=4, name=f"psc{n}")
            for k in range(TT):
                s = 896 - k * P + n * 512
                nc.tensor.matmul(psc[:], mask_bf[:, k * E:(k + 1) * E],
                                 tri[:, s:s + 512],
                                 start=(k == 0), stop=(k == TT - 1))
            nc.vector.tensor_copy(pos[:, n * (T // 2):(n + 1) * (T // 2)],
                                  psc[:])
        # maskT via PE transpose of mask_bf
        maskT = sbB.tile([E, T], BF16)
        for k in range(TT):
            ptm = pB(f"mtr{k}", BF16)
            nc.tensor.transpose(ptm[:E, :P], mask_bf[:, k * E:(k + 1) * E],
                                identb[:])
            nc.vector.tensor_copy(maskT[:, k * P:(k + 1) * P], ptm[:E, :P])
        # slot[e,t] = mask ? pos : C   == C + (pos - C) * mask
        nc.vector.tensor_scalar(pos[:], pos[:], float(C), scalar2=None,
                                op0=OP.subtract)
        nc.vector.tensor_tensor(out=pos[:], in0=pos[:], in1=maskT[:],
                                op=OP.mult)
        nc.vector.tensor_scalar(pos[:], pos[:], float(C), scalar2=None,
                                op0=OP.add)
        nc.vector.tensor_copy(slotT_bf[:], pos[:])
        # posTok: slot values token-major for the local experts
        for k in range(TT):
            ptt = pB(f"ptr{k}")
            nc.tensor.transpose(ptt[:, :E], pos[:, k * P:(k + 1) * P],
                                ident[:E, :E])
            nc.vector.tensor_copy(posTok[:, k * EL:(k + 1) * EL],
                                  ptt[:, 0:EL])

        # ---- prep one expert: srow, slot->token, gather, transpose ----
        def prep(e):
            # srow: broadcast slotT row e to all partitions
            srow = sbB.tile([P, T], BF16, tag="srow", bufs=2, name=f"srow{e}")
            for n in range(2):
                psb = pA(f"srow{e}{n}")
                nc.tensor.matmul(psb[:], sel4[0:E, e * P:(e + 1) * P],
                                 slotT_bf[0:E, n * 512:(n + 1) * 512],
                                 start=True, stop=True)
                nc.vector.tensor_copy(srow[:, n * 512:(n + 1) * 512], psb[:])
            for half in range(2):
                nc.vector.tensor_tensor(
                    out=gmat[e][:, half * T:(half + 1) * T],
                    in0=ihalf_bf[:, half:half + 1].to_broadcast([P, T]),
                    in1=srow[:], op=OP.is_equal)
            # slot -> (token id, weight): accumulate over t-tiles
            pstk = ps.tile([5, C], F32, tag="pB", bufs=4, name=f"stk{e}")
            for k in range(TT):
                ptk = sbB.tile([P, C], F32R, tag="petk", bufs=2,
                               name=f"petk{e}{k}")
                nc.vector.tensor_tensor(
                    out=ptk[:],
                    in0=posTok[:, k * EL + e:k * EL + e + 1].to_broadcast(
                        [P, C]),
                    in1=iota_c[:], op=OP.is_equal)
                nc.tensor.matmul(pstk[:], tok5[:, k * 5:(k + 1) * 5], ptk[:],
                                 start=(k == 0), stop=(k == TT - 1))
            stokw = sbB.tile([E, C], F32, tag="stokw", bufs=2,
                             name=f"stokw{e}")
            nc.vector.memset(stokw[:], 0.0)
            nc.vector.tensor_copy(stokw[0:5, :], pstk[:])
            stok_i = sbB.tile([P, 2], I32, tag="stoki", bufs=2,
                              name=f"stoki{e}")
            for half in range(2):
                ptt = pB(f"st{e}{half}")
                nc.tensor.transpose(ptt[:, 0:E],
                                    stokw[0:E, half * P:(half + 1) * P],
                                    ident[0:E, 0:E])
                nc.vector.tensor_copy(stok_i[:, half:half + 1], ptt[:, 0:1])
                nc.vector.tensor_copy(wslot[e][:, half:half + 1],
                                      ptt[:, 1 + e:2 + e])
            # gather x rows and transpose to [h, c]
            for half in range(2):
                xe = sbB.tile([P, H], BF16, tag="xe", bufs=2,
                              name=f"xe{e}{half}")
                nc.gpsimd.indirect_dma_start(
                    out=xe[:], out_offset=None, in_=xbf_d[:, :],
                    in_offset=bass.IndirectOffsetOnAxis(
                        ap=stok_i[:, half:half + 1], axis=0))
                for hc in range(HC):
                    ptx = pB(f"xt{e}{half}{hc}", BF16)
                    nc.tensor.transpose(ptx[:, :P],
                                        xe[:, hc * P:(hc + 1) * P],
                                        identb[:])
                    nc.vector.tensor_copy(
                        xet[e][:, hc * C + half * P:hc * C + half * P + P],
                        ptx[:, :P])

        s1_block(1)
        prep(0)
        prep(1)
        s1_block(2)
        prep(2)
        prep(3)

    # ================= phase 2: expert MLPs ===============================
    with tc.tile_pool(name="sbC", bufs=1) as sbC:
        ws2_sb = sbC.tile([P, 3 * H], BF16)
        nc.scalar.dma_start(out=ws2_sb[:], in_=ws2_d[:, :])
        # ---- MM1 + MM2 per expert ----
        for e in range(EL):
            act_e = sbC.tile([P, FT * C], BF16, tag="act", bufs=2,
                             name=f"act{e}")
            for m in range(FT):
                w1p = sbC.tile([P, HC * 2 * P], BF16, tag="w1p", bufs=4,
                               name=f"w1p{e}{m}")
                nc.sync.dma_start(out=w1p[:], in_=w1_d[e, m, :, :])
                psg = pB(f"mg{e}{m}")
                psu = pB(f"mu{e}{m}")
                for k in range(HC):
                    rhs = xet[e][:, k * C:(k + 1) * C]
                    nc.tensor.matmul(psg[:], w1p[:, k * 2 * P:k * 2 * P + P],
                                     rhs, start=(k == 0), stop=(k == HC - 1))
                    nc.tensor.matmul(psu[:],
                                     w1p[:, k * 2 * P + P:(k + 1) * 2 * P],
                                     rhs, start=(k == 0), stop=(k == HC - 1))
                sgt = sbC.tile([P, C], F32, tag="sgt", bufs=2,
                               name=f"sgt{e}{m}")
                nc.scalar.activation(sgt[:], psg[:], AF.Sigmoid)
                nc.vector.tensor_tensor(out=sgt[:], in0=sgt[:], in1=psg[:],
                                        op=OP.mult)
                nc.vector.tensor_tensor(out=act_e[:, m * C:(m + 1) * C],
                                        in0=sgt[:], in1=psu[:], op=OP.mult)
            for n in range(4):
                w2t = sbC.tile([P, FT * 512], BF16, tag="w2t", bufs=2,
                               name=f"w2t{e}{n}")
                nc.sync.dma_start(out=w2t[:], in_=w2_d[e, n, :, :])
                psy = [pA(f"y{e}{n}{mc}") for mc in range(2)]
                for kf in range(FT):
                    for mc in range(2):
                        nc.tensor.matmul(
                            psy[mc][:],
                            act_e[:, kf * C + mc * P:kf * C + (mc + 1) * P],
                            w2t[:, kf * 512:(kf + 1) * 512],
                            start=(kf == 0), stop=(kf == FT - 1))
                for mc in range(2):
                    nc.vector.tensor_scalar_mul(
                        y_sb[e][:, mc * H + n * 512:mc * H + (n + 1) * 512],
                        psy[mc][:], wslot[e][:, mc:mc + 1])

        # ================= phase 3: combine + ReduceScatter ===============
        for hh in range(2):
            for mt in range(TT):
                for nn in range(2):
                    pc = pA(f"c{hh}{mt}{nn}")
                    col = hh * 1024 + nn * 512
                    for kf in range(3):
                        nc.tensor.matmul(
                            pc[:],
                            act_sT[:, kf * T + mt * P:kf * T + (mt + 1) * P],
                            ws2_sb[:, kf * H + col:kf * H + col + 512],
                            start=(kf == 0), stop=False)
                    cnt = 3
                    for e in range(EL):
                        for ch in range(2):
                            cnt += 1
                            nc.tensor.matmul(
                                pc[:],
                                gmat[e][:, ch * T + mt * P:
                                        ch * T + (mt + 1) * P],
                                y_sb[e][:, ch * H + col:ch * H + col + 512],
                                start=False, stop=(cnt == 11))
                    ob = sbC.tile([P, 512], BF16, tag="ob", bufs=4,
                                  name=f"ob{hh}{mt}{nn}")
                    nc.vector.tensor_copy(ob[:], pc[:])
                    nc.sync.dma_start(
                        out=acc_d[hh][mt * P:(mt + 1) * P,
                                      nn * 512:(nn + 1) * 512],
                        in_=ob[:])
            nc.gpsimd.collective_compute(
                "ReduceScatter", OP.add,
                replica_groups=[list(range(NCORES))],
                ins=[acc_d[hh][:, :]], outs=[rs_d[hh][:, :]])
            nc.scalar.dma_start(out=out_d[:, hh * 1024:(hh + 1) * 1024],
                                in_=rs_d[hh][:, :])

    ctx.close()


# ---------------- host side ----------------
_CACHED = {}


def _get_program():
    if "nc" not in _CACHED:
        _CACHED["nc"] = build_program()
    return _CACHED["nc"]


def make_in_maps(hidden_states, w_gate, w1, w2, ws1, ws2):
    bf = ml_dtypes.bfloat16
    x = np.ascontiguousarray(hidden_states, dtype=np.float32)
    xT = np.ascontiguousarray(x.T)                                 # [H, T]
    xbfT = np.ascontiguousarray(
        xT.reshape(HC, P, T).transpose(1, 0, 2).reshape(P, HC * T)).astype(bf)
    xbf = x.astype(bf)                                             # [T, H]
    wg = np.asarray(w_gate, np.float32)                            # [E, H]
    w1 = np.asarray(w1, np.float32)
    w2 = np.asarray(w2, np.float32)
    ws1 = np.asarray(ws1, np.float32)
    ws2 = np.asarray(ws2, np.float32)

    # w1 interleaved (gate_m | up_m) then tiled [E, FT, P, HC*256]
    w1c = np.concatenate([w1[:, :, :F].reshape(E, H, FT, P),
                          w1[:, :, F:].reshape(E, H, FT, P)], axis=3)
    w1t = np.ascontiguousarray(
        w1c.reshape(E, HC, P, FT, 2 * P).transpose(0, 3, 2, 1, 4).reshape(
            E, FT, P, HC * 2 * P)).astype(bf)
    # w2 tiled [E, 4, P, FT*512]
    w2t = np.ascontiguousarray(
        w2.reshape(E, FT, P, 4, 512).transpose(0, 3, 2, 1, 4).reshape(
            E, 4, P, FT * 512)).astype(bf)

    shard = FS // NCORES  # 352
    in_maps = []
    for k in range(NCORES):
        # shared expert shard, padded 352 -> 384, tiled [P, HC*768]
        ws1p = np.zeros((H, 2 * SSH), np.float32)
        ws1p[:, :shard] = ws1[:, k * shard:(k + 1) * shard]
        ws1p[:, SSH:SSH + shard] = ws1[:, FS + k * shard:FS + (k + 1) * shard]
        ws1l = np.ascontiguousarray(
            ws1p.reshape(HC, P, 2 * SSH).transpose(1, 0, 2).reshape(
                P, HC * 768)).astype(bf)
        ws2p = np.zeros((SSH, H), np.float32)
        ws2p[:shard] = ws2[k * shard:(k + 1) * shard]
        ws2l = np.ascontiguousarray(
            ws2p.reshape(3, P, H).transpose(1, 0, 2).reshape(
                P, 3 * H)).astype(bf)
        # group permutation: swap group 0 <-> group k so the core's own
        # 4 experts are rows 0..3 (grouped top-k is group-order invariant)
        gperm = list(range(G_GRP))
        gperm[0], gperm[k] = gperm[k], gperm[0]
        eperm = [g * EL + i for g in gperm for i in range(EL)]
        wgp = wg[eperm]                                            # [E, H]
        wgt = np.ascontiguousarray(
            wgp.T.reshape(HC, P, E).transpose(1, 0, 2).reshape(
                P, HC * E)).astype(np.float32)
        in_maps.append({
            "xT": xT,
            "xbfT": xbfT,
            "xbf": xbf,
            "wgt": wgt,
            "w1l": np.ascontiguousarray(w1t[k * EL:(k + 1) * EL]),
            "w2l": np.ascontiguousarray(w2t[k * EL:(k + 1) * EL]),
            "ws1l": ws1l,
            "ws2l": ws2l,
        })
    return in_maps


def kernel(hidden_states, w_gate, w1, w2, ws1, ws2):
    from concourse.bass_utils import run_bass_kernel_spmd
    nc = _get_program()
    in_maps = make_in_maps(hidden_states, w_gate, w1, w2, ws1, ws2)
    res = run_bass_kernel_spmd(nc, in_maps, list(range(NCORES)))
    shards = [res.results[k]["out"] for k in range(NCORES)]
    return np.concatenate(shards, axis=0).astype(np.float32)


# revision 27
# speedup vs baseline: 2.9288x; 1.0082x over previous
"""DeepseekV2 MoE layer on 8 TRN2 NeuronCores (expert-parallel).

Sharding: w1/w2 sharded 4-experts-per-core; router + token activations
replicated; shared expert tensor-parallel along the FS dim (352/core,
zero-padded to 384). Routing (softmax + grouped top-k) computed on device.

Per core: router logits (f32r matmul) -> grouped top-k -> per-expert slot
assignment via triangular-matmul cumsum -> indirect-DMA token gather ->
capacity-C expert MLP in bf16 -> one-hot combine matmul (weights folded
into y) accumulated with the shared-expert down-proj in PSUM -> bf16
ReduceScatter in two h-halves -> each core emits its 128 output rows.

Expert groups are host-permuted per core so the core's own 4 experts are
always rows 0-3 of the routing tensors (group-swap is transparent to the
grouped top-k). All weights are host-pre-tiled into the exact SBUF layout
so each weight load is a single large-descriptor DMA on the sync HWDGE
queue.
"""

import numpy as np
import ml_dtypes

import concourse.bass as bass
import concourse.tile as tile
from concourse import bacc, mybir
from concourse.masks import make_identity

# problem shape
T, H = 1024, 2048
E, F = 32, 1408
F2 = 2 * F                      # 2816
G_GRP, TOPK_G, TOPK = 8, 3, 6
FS = 2 * F                      # 2816 shared intermediate
SCALE = 16.0
NCORES = 8
EL = E // NCORES                # 4 experts per core
C = 256                         # per-expert token capacity (max seen ~212)
P = 128
TT = T // P                     # 8 token tiles
HC = H // P                     # 16 h chunks
FT = F // P                     # 11 f tiles
SSH = 384                       # padded shared shard (352 real)
TRIW = 1920                     # cumsum window width

F32 = mybir.dt.float32
F32R = mybir.dt.float32r
BF16 = mybir.dt.bfloat16
F16 = mybir.dt.float16
I32 = mybir.dt.int32
AF = mybir.ActivationFunctionType
OP = mybir.AluOpType


def build_program():
    nc = bacc.Bacc("TRN2", target_bir_lowering=False, debug=False,
                   num_devices=NCORES)

    xbfT_d = nc.dram_tensor("xbfT", [P, HC * T], BF16, kind="ExternalInput")
    xrT_d = nc.dram_tensor("xrT", [P, HC * T], BF16, kind="ExternalInput")
    xbf_d = nc.dram_tensor("xbf", [T, H], BF16, kind="ExternalInput")
    wg_d = nc.dram_tensor("wgt", [P, 2 * HC * E], BF16, kind="ExternalInput")
    w1_d = nc.dram_tensor("w1l", [EL, FT, P, HC * 2 * P], BF16,
                          kind="ExternalInput")
    w2_d = nc.dram_tensor("w2l", [EL, 4, P, FT * 512], BF16,
                          kind="ExternalInput")
    ws1_d = nc.dram_tensor("ws1l", [P, HC * 768], BF16, kind="ExternalInput")
    ws2_d = nc.dram_tensor("ws2l", [P, 3 * H], BF16, kind="ExternalInput")
    out_d = nc.dram_tensor("out", [P, H], BF16, kind="ExternalOutput")

    acc_d = [nc.dram_tensor(f"acc{i}", [T, H // 2], BF16) for i in range(2)]
    rs_d = [nc.dram_tensor(f"rs{i}", [P, H // 2], BF16) for i in range(2)]

    with tile.TileContext(nc) as tc:
        _build(nc, tc, xbfT_d, xrT_d, xbf_d, wg_d, w1_d, w2_d, ws1_d, ws2_d,
               out_d, acc_d, rs_d)

    nc.compile()
    return nc


def _build(nc, tc, xbfT_d, xrT_d, xbf_d, wg_d, w1_d, w2_d, ws1_d, ws2_d,
           out_d, acc_d, rs_d):
    import contextlib
    ctx = contextlib.ExitStack()
    sbA = ctx.enter_context(tc.tile_pool(name="sbA", bufs=1))
    ps = ctx.enter_context(tc.tile_pool(name="ps", bufs=1, space="PSUM"))

    def pA(name):
        return ps.tile([P, 512], F32, tag="pA", bufs=4, name=name)

    def pB(name, dt=F32):
        return ps.tile([P, C], dt, tag="pB", bufs=4, name=name)

    # ---- constants ----
    ident = sbA.tile([P, P], F32)
    make_identity(nc, ident[:])
    identb = sbA.tile([P, P], BF16)
    make_identity(nc, identb[:])
    iota_ci = sbA.tile([P, C], I32)
    nc.gpsimd.iota(iota_ci[:], pattern=[[1, C]], base=0, channel_multiplier=0)
    iota_c = sbA.tile([P, C], F32)
    nc.vector.tensor_copy(iota_c[:], iota_ci[:])
    ihalf_i = sbA.tile([P, 2], I32)      # col h: value 128*h + p
    nc.gpsimd.iota(ihalf_i[:], pattern=[[P, 2]], base=0, channel_multiplier=1)
    ihalf_bf = sbA.tile([P, 2], BF16)
    nc.vector.tensor_copy(ihalf_bf[:], ihalf_i[:])
    toki = sbA.tile([P, TT], I32)        # col k: value 128*k + p
    nc.gpsimd.iota(toki[:], pattern=[[P, TT]], base=0, channel_multiplier=1)
    tok5 = sbA.tile([P, TT * 5], F32R)   # per k: [tok | w_e0..w_e3]
    tok5v = tok5[:].rearrange("p (k c) -> p k c", c=5)
    nc.vector.tensor_copy(tok5v[:, :, 0:1], toki[:].rearrange(
        "p (k c) -> p k c", c=1))
    tri = sbA.tile([P, TRIW], BF16)      # tri[p, q] = (q >= p + 897)
    nc.vector.memset(tri[:], 1.0)
    nc.gpsimd.affine_select(out=tri[:], in_=tri[:], pattern=[[1, TRIW]],
                            compare_op=OP.is_ge, fill=0.0, base=-897,
                            channel_multiplier=-1)
    sel4 = sbA.tile([P, EL * P], BF16)   # [k, e*128+m] = (k == e)
    nc.vector.memset(sel4[:], 1.0)
    nc.gpsimd.affine_select(out=sel4[:], in_=sel4[:], pattern=[[1, EL * P]],
                            compare_op=OP.is_ge, fill=0.0, base=0,
                            channel_multiplier=-P)
    nc.gpsimd.affine_select(out=sel4[:], in_=sel4[:], pattern=[[-1, EL * P]],
                            compare_op=OP.is_ge, fill=0.0, base=P - 1,
                            channel_multiplier=P)

    # ---- persistent tiles ----
    act_sT = sbA.tile([P, 3 * T], BF16)          # shared act^T [fs, t]
    slotT_bf = sbA.tile([E, T], BF16)
    posTok = sbA.tile([P, TT * EL], F32)         # slot of token, local experts
    y_sb = [sbA.tile([P, 2 * H], BF16, tag=f"y{e}", name=f"y{e}")
            for e in range(EL)]
    gmat = [sbA.tile([P, 2 * T], BF16, tag=f"g{e}", name=f"gm{e}")
            for e in range(EL)]
    wslot = [sbA.tile([P, 2], F32, tag=f"w{e}", name=f"ws{e}")
             for e in range(EL)]
    xet = [sbA.tile([P, HC * C], BF16, tag=f"xet{e}", name=f"xet{e}")
           for e in range(EL)]

    # ================= phase 1: router + shared up-proj + routing =========
    with tc.tile_pool(name="sbB", bufs=1) as sbB:
        wg_sb = sbB.tile([P, 2 * HC * E], BF16)   # [wg_bf | wg_res] tiled
        nc.sync.dma_start(out=wg_sb[:], in_=wg_d[:, :])

        xbfT = sbB.tile([P, HC * T], BF16)
        for q in range(8):
            w = HC * T // 8
            nc.scalar.dma_start(out=xbfT[:, q * w:(q + 1) * w],
                                in_=xbfT_d[:, q * w:(q + 1) * w])
        ws1_sb = sbB.tile([P, HC * 768], BF16)
        nc.scalar.dma_start(out=ws1_sb[:], in_=ws1_d[:, :])

        logT = sbB.tile([E, T], F32)
        # router: logitsT = wg.T @ x.T in bf16 with residual correction:
        # wb.xb + wb.xr + wr.xb  (error ~1e-5, exceeds fp32-routing fidelity)
        psl = [ps.tile([E, T // 2], F32, tag="pA", bufs=4, name=f"psl{n}")
               for n in range(2)]
        for k in range(HC):
            if k % 2 == 0:
                xr = sbB.tile([P, 2 * T], BF16, tag="xr", bufs=2,
                              name=f"xr{k}")
                nc.sync.dma_start(out=xr[:],
                                  in_=xrT_d[:, k * T:(k + 2) * T])
            wb = wg_sb[:, k * E:(k + 1) * E]
            wr = wg_sb[:, (HC + k) * E:(HC + k + 1) * E]
            for n in range(2):
                xbk = xbfT[:, k * T + n * 512:k * T + (n + 1) * 512]
                xrk = xr[:, (k % 2) * T + n * 512:(k % 2) * T + (n + 1) * 512]
                nc.tensor.matmul(psl[n][:], wb, xbk,
                                 start=(k == 0), stop=False)
                nc.tensor.matmul(psl[n][:], wb, xrk, start=False, stop=False)
                nc.tensor.matmul(psl[n][:], wr, xbk, start=False,
                                 stop=(k == HC - 1))
        for n in range(2):
            nc.vector.tensor_copy(logT[:, n * (T // 2):(n + 1) * (T // 2)],
                                  psl[n][:])

        # transpose logitsT -> scores [128, (k e)]
        scores = sbB.tile([P, TT * E], F32)
        for k in range(TT):
            pst = pB(f"ltr{k}")
            nc.tensor.transpose(pst[:, :E], logT[:, k * P:(k + 1) * P],
                                ident[:E, :E])
            nc.vector.tensor_copy(scores[:, k * E:(k + 1) * E], pst[:, :E])

        # shared expert up-proj: act_sT[fs, t] (3 fs-tiles x 2 t-halves),
        # emitted in blocks interleaved with routing tail + expert prep so
        # the PE stays busy while routing/gather dependencies resolve
        def s1_block(mg):
            for n in range(2):
                psg = pA(f"psg{mg}{n}")
                psu = pA(f"psu{mg}{n}")
                for k in range(HC):
                    rhs = xbfT[:, k * T + n * 512:k * T + (n + 1) * 512]
                    nc.tensor.matmul(psg[:],
                                     ws1_sb[:, k * 768 + mg * P:
                                            k * 768 + (mg + 1) * P],
                                     rhs, start=(k == 0), stop=(k == HC - 1))
                    nc.tensor.matmul(psu[:],
                                     ws1_sb[:, k * 768 + SSH + mg * P:
                                            k * 768 + SSH + (mg + 1) * P],
                                     rhs, start=(k == 0), stop=(k == HC - 1))
                gsil = sbB.tile([P, 512], F32, tag="gsil", bufs=2,
                                name=f"gsil{mg}{n}")
                nc.scalar.activation(gsil[:], psg[:], AF.Sigmoid)
                nc.vector.tensor_tensor(out=gsil[:], in0=gsil[:], in1=psg[:],
                                        op=OP.mult)
                nc.vector.tensor_tensor(
                    out=act_sT[:, mg * T + n * 512:mg * T + (n + 1) * 512],
                    in0=gsil[:], in1=psu[:], op=OP.mult)

        s1_block(0)

        # softmax + grouped top-k (per t-tile)
        comb = sbB.tile([P, TT * E], F32)
        mask_bf = sbB.tile([P, TT * E], BF16)
        tmp8 = sbB.tile([P, 8], F32)
        for k in range(TT):
            blk = scores[:, k * E:(k + 1) * E]
            mx = sbB.tile([P, 1], F32, tag="rmax", bufs=2, name=f"rmax{k}")
            nc.vector.tensor_reduce(mx[:], blk, axis=mybir.AxisListType.X,
                                    op=OP.max, negate=True)
            sm = sbB.tile([P, 1], F32, tag="rsum", bufs=2, name=f"rsum{k}")
            nc.scalar.activation(blk, blk, AF.Exp, bias=mx[:], accum_out=sm[:])
            rc = sbB.tile([P, 1], F32, tag="rrec", bufs=2, name=f"rrec{k}")
            nc.vector.reciprocal(rc[:], sm[:])
            nc.vector.tensor_scalar_mul(blk, blk, rc[:])

            blk3 = blk.rearrange("p (g f) -> p g f", f=4)
            gsc = sbB.tile([P, G_GRP], F32, tag="gsc", bufs=2, name=f"gsc{k}")
            nc.vector.tensor_reduce(gsc[:], blk3, axis=mybir.AxisListType.X,
                                    op=OP.max)
            nc.vector.max(out=tmp8[:], in_=gsc[:])
            nc.vector.memset(tmp8[:, TOPK_G:], 0.0)
            gz = sbB.tile([P, G_GRP], F32, tag="gz", bufs=2, name=f"gz{k}")
            nc.vector.match_replace(out=gz[:], in_to_replace=tmp8[:],
                                    in_values=gsc[:], imm_value=0.0)
            nc.vector.tensor_tensor(out=gz[:], in0=gsc[:], in1=gz[:],
                                    op=OP.subtract)
            nc.vector.tensor_scalar(gz[:], gz[:], 0.0, scalar2=None,
                                    op0=OP.is_gt)
            cblk = comb[:, k * E:(k + 1) * E]
            cblk3 = cblk.rearrange("p (g f) -> p g f", f=4)
            gz3 = gz[:].rearrange("p (g o) -> p g o", o=1)
            nc.vector.tensor_tensor(out=cblk3, in0=blk3,
                                    in1=gz3.to_broadcast([P, G_GRP, 4]),
                                    op=OP.mult)
            nc.vector.max(out=tmp8[:], in_=cblk)
            nc.vector.memset(tmp8[:, TOPK:], 0.0)
            zap = sbB.tile([P, E], F32, tag="zap", bufs=2, name=f"zap{k}")
            nc.vector.match_replace(out=zap[:], in_to_replace=tmp8[:],
                                    in_values=cblk, imm_value=0.0)
            nc.vector.tensor_tensor(out=cblk, in0=cblk, in1=zap[:],
                                    op=OP.subtract)
            nc.vector.tensor_scalar_mul(cblk, cblk, SCALE)
            nc.vector.tensor_scalar(mask_bf[:, k * E:(k + 1) * E], cblk, 0.0,
                                    scalar2=None, op0=OP.is_gt)
        # local-expert weights into the slot-matmul lhsT (cols 1..4 per k)
        nc.vector.tensor_copy(
            tok5v[:, :, 1:5],
            comb[:].rearrange("p (k e) -> p k e", e=E)[:, :, 0:EL])

        # strict cumsum: pos[e, t] = sum_{t' < t} mask[e, t']
        pos = sbB.tile([E, T], F32)
        for n in range(2):
            psc = ps.tile([E, T // 2], F32, tag="pA", bufs=4, name=f"psc{n}")
            for k in range(TT):
                s = 896 - k * P + n * 512
                nc.tensor.matmul(psc[:], mask_bf[:, k * E:(k + 1) * E],
                                 tri[:, s:s + 512],
                                 start=(k == 0), stop=(k == TT - 1))
            nc.vector.tensor_copy(pos[:, n * (T // 2):(n + 1) * (T // 2)],
                                  psc[:])
        # maskT via PE transpose of mask_bf
        maskT = sbB.tile([E, T], BF16)
        for k in range(TT):
            ptm = pB(f"mtr{k}", BF16)
            nc.tensor.transpose(ptm[:E, :P], mask_bf[:, k * E:(k + 1) * E],
                                identb[:])
            nc.vector.tensor_copy(maskT[:, k * P:(k + 1) * P], ptm[:E, :P])
        # slot[e,t] = mask ? pos : C   == C + (pos - C) * mask
        nc.vector.tensor_scalar(pos[:], pos[:], float(C), scalar2=None,
                                op0=OP.subtract)
        nc.vector.tensor_tensor(out=pos[:], in0=pos[:], in1=maskT[:],
                                op=OP.mult)
        nc.vector.tensor_scalar(pos[:], pos[:], float(C), scalar2=None,
                                op0=OP.add)
        nc.vector.tensor_copy(slotT_bf[:], pos[:])
        # posTok: slot values token-major for the local experts
        for k in range(TT):
            ptt = pB(f"ptr{k}")
            nc.tensor.transpose(ptt[:, :E], pos[:, k * P:(k + 1) * P],
                                ident[:E, :E])
            nc.vector.tensor_copy(posTok[:, k * EL:(k + 1) * EL],
                                  ptt[:, 0:EL])

        # ---- prep one expert: srow, slot->token, gather, transpose ----
        def prep(e):
            # srow: broadcast slotT row e to all partitions
            srow = sbB.tile([P, T], BF16, tag="srow", bufs=2, name=f"srow{e}")
            for n in range(2):
                psb = pA(f"srow{e}{n}")
                nc.tensor.matmul(psb[:], sel4[0:E, e * P:(e + 1) * P],
                                 slotT_bf[0:E, n * 512:(n + 1) * 512],
                                 start=True, stop=True)
                nc.vector.tensor_copy(srow[:, n * 512:(n + 1) * 512], psb[:])
            for half in range(2):
                nc.vector.tensor_tensor(
                    out=gmat[e][:, half * T:(half + 1) * T],
                    in0=ihalf_bf[:, half:half + 1].to_broadcast([P, T]),
                    in1=srow[:], op=OP.is_equal)
            # slot -> (token id, weight): accumulate over t-tiles
            pstk = ps.tile([5, C], F32, tag="pB", bufs=4, name=f"stk{e}")
            for k in range(TT):
                ptk = sbB.tile([P, C], F32R, tag="petk", bufs=2,
                               name=f"petk{e}{k}")
                nc.vector.tensor_tensor(
                    out=ptk[:],
                    in0=posTok[:, k * EL + e:k * EL + e + 1].to_broadcast(
                        [P, C]),
                    in1=iota_c[:], op=OP.is_equal)
                nc.tensor.matmul(pstk[:], tok5[:, k * 5:(k + 1) * 5], ptk[:],
                                 start=(k == 0), stop=(k == TT - 1))
            stokw = sbB.tile([E, C], F32, tag="stokw", bufs=2,
                             name=f"stokw{e}")
            nc.vector.memset(stokw[:], 0.0)
            nc.vector.tensor_copy(stokw[0:5, :], pstk[:])
            stok_i = sbB.tile([P, 2], I32, tag="stoki", bufs=2,
                              name=f"stoki{e}")
            for half in range(2):
                ptt = pB(f"st{e}{half}")
                nc.tensor.transpose(ptt[:, 0:E],
                                    stokw[0:E, half * P:(half + 1) * P],
                                    ident[0:E, 0:E])
                nc.vector.tensor_copy(stok_i[:, half:half + 1], ptt[:, 0:1])
                nc.vector.tensor_copy(wslot[e][:, half:half + 1],
                                      ptt[:, 1 + e:2 + e])
            # gather x rows, then XBAR DMA-transpose to [h, c] layout
            for half in range(2):
                xe = sbB.tile([P, H], BF16, tag="xe", bufs=2,
                              name=f"xe{e}{half}")
                nc.gpsimd.indirect_dma_start(
                    out=xe[:], out_offset=None, in_=xbf_d[:, :],
                    in_offset=bass.IndirectOffsetOnAxis(
                        ap=stok_i[:, half:half + 1], axis=0))
                outap = xet[e][:].rearrange("p (hc c) -> p hc c", hc=HC)[
                    :, :, half * P:half * P + P]
                nc.scalar.dma_start(out=outap, in_=xe[:], transpose=True)

        s1_block(1)
        prep(0)
        prep(1)
        s1_block(2)
        prep(2)
        prep(3)

    # ================= phase 2: expert MLPs ===============================
    with tc.tile_pool(name="sbC", bufs=1) as sbC:
        ws2_sb = sbC.tile([P, 3 * H], BF16)
        nc.scalar.dma_start(out=ws2_sb[:], in_=ws2_d[:, :])
        # ---- MM1 + MM2 per expert ----
        for e in range(EL):
            act_e = sbC.tile([P, FT * C], BF16, tag="act", bufs=2,
                             name=f"act{e}")
            for m in range(FT):
                w1p = sbC.tile([P, HC * 2 * P], BF16, tag="w1p", bufs=4,
                               name=f"w1p{e}{m}")
                nc.sync.dma_start(out=w1p[:], in_=w1_d[e, m, :, :])
                psg = pB(f"mg{e}{m}")
                psu = pB(f"mu{e}{m}")
                for k in range(HC):
                    rhs = xet[e][:, k * C:(k + 1) * C]
                    nc.tensor.matmul(psg[:], w1p[:, k * 2 * P:k * 2 * P + P],
                                     rhs, start=(k == 0), stop=(k == HC - 1))
                    nc.tensor.matmul(psu[:],
                                     w1p[:, k * 2 * P + P:(k + 1) * 2 * P],
                                     rhs, start=(k == 0), stop=(k == HC - 1))
                sgt = sbC.tile([P, C], F32, tag="sgt", bufs=2,
                               name=f"sgt{e}{m}")
                nc.scalar.activation(sgt[:], psg[:], AF.Sigmoid)
                nc.vector.tensor_tensor(out=sgt[:], in0=sgt[:], in1=psg[:],
                                        op=OP.mult)
                nc.vector.tensor_tensor(out=act_e[:, m * C:(m + 1) * C],
                                        in0=sgt[:], in1=psu[:], op=OP.mult)
            for n in range(4):
                w2t = sbC.tile([P, FT * 512], BF16, tag="w2t", bufs=2,
                               name=f"w2t{e}{n}")
                nc.sync.dma_start(out=w2t[:], in_=w2_d[e, n, :, :])
                psy = [pA(f"y{e}{n}{mc}") for mc in range(2)]
                for kf in range(FT):
                    for mc in range(2):
                        nc.tensor.matmul(
                            psy[mc][:],
                            act_e[:, kf * C + mc * P:kf * C + (mc + 1) * P],
                            w2t[:, kf * 512:(kf + 1) * 512],
                            start=(kf == 0), stop=(kf == FT - 1))
                for mc in range(2):
                    nc.vector.tensor_scalar_mul(
                        y_sb[e][:, mc * H + n * 512:mc * H + (n + 1) * 512],
                        psy[mc][:], wslot[e][:, mc:mc + 1])

        # ================= phase 3: combine + ReduceScatter ===============
        for hh in range(2):
            for mt in range(TT):
                for nn in range(2):
                    pc = pA(f"c{hh}{mt}{nn}")
                    col = hh * 1024 + nn * 512
                    for kf in range(3):
                        nc.tensor.matmul(
                            pc[:],
                            act_sT[:, kf * T + mt * P:kf * T + (mt + 1) * P],
                            ws2_sb[:, kf * H + col:kf * H + col + 512],
                            start=(kf == 0), stop=False)
                    cnt = 3
                    for e in range(EL):
                        for ch in range(2):
                            cnt += 1
                            nc.tensor.matmul(
                                pc[:],
                                gmat[e][:, ch * T + mt * P:
                                        ch * T + (mt + 1) * P],
                                y_sb[e][:, ch * H + col:ch * H + col + 512],
                                start=False, stop=(cnt == 11))
                    ob = sbC.tile([P, 512], BF16, tag="ob", bufs=4,
                                  name=f"ob{hh}{mt}{nn}")
                    nc.vector.tensor_copy(ob[:], pc[:])
                    nc.sync.dma_start(
                        out=acc_d[hh][mt * P:(mt + 1) * P,
                                      nn * 512:(nn + 1) * 512],
                        in_=ob[:])
            nc.gpsimd.collective_compute(
                "ReduceScatter", OP.add,
                replica_groups=[list(range(NCORES))],
                ins=[acc_d[hh][:, :]], outs=[rs_d[hh][:, :]])
            nc.scalar.dma_start(out=out_d[:, hh * 1024:(hh + 1) * 1024],
                                in_=rs_d[hh][:, :])

    ctx.close()


# ---------------- host side ----------------
_CACHED = {}


def _get_program():
    if "nc" not in _CACHED:
        _CACHED["nc"] = build_program()
    return _CACHED["nc"]


def make_in_maps(hidden_states, w_gate, w1, w2, ws1, ws2):
    bf = ml_dtypes.bfloat16
    x = np.ascontiguousarray(hidden_states, dtype=np.float32)
    xTt = np.ascontiguousarray(
        x.T.reshape(HC, P, T).transpose(1, 0, 2).reshape(P, HC * T))
    xbfT = xTt.astype(bf)
    xrT = (xTt - xbfT.astype(np.float32)).astype(bf)
    xbf = x.astype(bf)                                             # [T, H]
    wg = np.asarray(w_gate, np.float32)                            # [E, H]
    w1 = np.asarray(w1, np.float32)
    w2 = np.asarray(w2, np.float32)
    ws1 = np.asarray(ws1, np.float32)
    ws2 = np.asarray(ws2, np.float32)

    # w1 interleaved (gate_m | up_m) then tiled [E, FT, P, HC*256]
    w1c = np.concatenate([w1[:, :, :F].reshape(E, H, FT, P),
                          w1[:, :, F:].reshape(E, H, FT, P)], axis=3)
    w1t = np.ascontiguousarray(
        w1c.reshape(E, HC, P, FT, 2 * P).transpose(0, 3, 2, 1, 4).reshape(
            E, FT, P, HC * 2 * P)).astype(bf)
    # w2 tiled [E, 4, P, FT*512]
    w2t = np.ascontiguousarray(
        w2.reshape(E, FT, P, 4, 512).transpose(0, 3, 2, 1, 4).reshape(
            E, 4, P, FT * 512)).astype(bf)

    shard = FS // NCORES  # 352
    in_maps = []
    for k in range(NCORES):
        # shared expert shard, padded 352 -> 384, tiled [P, HC*768]
        ws1p = np.zeros((H, 2 * SSH), np.float32)
        ws1p[:, :shard] = ws1[:, k * shard:(k + 1) * shard]
        ws1p[:, SSH:SSH + shard] = ws1[:, FS + k * shard:FS + (k + 1) * shard]
        ws1l = np.ascontiguousarray(
            ws1p.reshape(HC, P, 2 * SSH).transpose(1, 0, 2).reshape(
                P, HC * 768)).astype(bf)
        ws2p = np.zeros((SSH, H), np.float32)
        ws2p[:shard] = ws2[k * shard:(k + 1) * shard]
        ws2l = np.ascontiguousarray(
            ws2p.reshape(3, P, H).transpose(1, 0, 2).reshape(
                P, 3 * H)).astype(bf)
        # group permutation: swap group 0 <-> group k so the core's own
        # 4 experts are rows 0..3 (grouped top-k is group-order invariant)
        gperm = list(range(G_GRP))
        gperm[0], gperm[k] = gperm[k], gperm[0]
        eperm = [g * EL + i for g in gperm for i in range(EL)]
        wgp = wg[eperm]                                            # [E, H]
        wgtf = np.ascontiguousarray(
            wgp.T.reshape(HC, P, E).transpose(1, 0, 2).reshape(P, HC * E))
        wgb = wgtf.astype(bf)
        wgr = (wgtf - wgb.astype(np.float32)).astype(bf)
        in_maps.append({
            "xbfT": xbfT,
            "xrT": xrT,
            "xbf": xbf,
            "wgt": np.ascontiguousarray(
                np.concatenate([wgb, wgr], axis=1)),
            "w1l": np.ascontiguousarray(w1t[k * EL:(k + 1) * EL]),
            "w2l": np.ascontiguousarray(w2t[k * EL:(k + 1) * EL]),
            "ws1l": ws1l,
            "ws2l": ws2l,
        })
    return in_maps


def kernel(hidden_states, w_gate, w1, w2, ws1, ws2):
    from concourse.bass_utils import run_bass_kernel_spmd
    nc = _get_program()
    in_maps = make_in_maps(hidden_states, w_gate, w1, w2, ws1, ws2)
    res = run_bass_kernel_spmd(nc, in_maps, list(range(NCORES)))
    shards = [res.results[k]["out"] for k in range(NCORES)]
    return np.concatenate(shards, axis=0).astype(np.float32)


# revision 30
# speedup vs baseline: 2.9721x; 1.0148x over previous
"""DeepseekV2 MoE layer on 8 TRN2 NeuronCores (expert-parallel).

Sharding: w1/w2 sharded 4-experts-per-core; router + token activations
replicated; shared expert tensor-parallel along the FS dim (352/core,
zero-padded to 384). Routing (softmax + grouped top-k) computed on device.

Per core: router logits (f32r matmul) -> grouped top-k -> per-expert slot
assignment via triangular-matmul cumsum -> indirect-DMA token gather ->
capacity-C expert MLP in bf16 -> one-hot combine matmul (weights folded
into y) accumulated with the shared-expert down-proj in PSUM -> bf16
ReduceScatter in two h-halves -> each core emits its 128 output rows.

Expert groups are host-permuted per core so the core's own 4 experts are
always rows 0-3 of the routing tensors (group-swap is transparent to the
grouped top-k). All weights are host-pre-tiled into the exact SBUF layout
so each weight load is a single large-descriptor DMA on the sync HWDGE
queue.
"""

import numpy as np
import ml_dtypes

import concourse.bass as bass
import concourse.tile as tile
from concourse import bacc, mybir
from concourse.masks import make_identity

# problem shape
T, H = 1024, 2048
E, F = 32, 1408
F2 = 2 * F                      # 2816
G_GRP, TOPK_G, TOPK = 8, 3, 6
FS = 2 * F                      # 2816 shared intermediate
SCALE = 16.0
NCORES = 8
EL = E // NCORES                # 4 experts per core
C = 256                         # per-expert token capacity (max seen ~212)
P = 128
TT = T // P                     # 8 token tiles
HC = H // P                     # 16 h chunks
FT = F // P                     # 11 f tiles
SSH = 384                       # padded shared shard (352 real)
TRIW = 1920                     # cumsum window width

F32 = mybir.dt.float32
F32R = mybir.dt.float32r
BF16 = mybir.dt.bfloat16
F16 = mybir.dt.float16
I32 = mybir.dt.int32
AF = mybir.ActivationFunctionType
OP = mybir.AluOpType


def build_program():
    nc = bacc.Bacc("TRN2", target_bir_lowering=False, debug=False,
                   num_devices=NCORES)

    xbfT_d = nc.dram_tensor("xbfT", [P, HC * T], BF16, kind="ExternalInput")
    xrT_d = nc.dram_tensor("xrT", [P, HC * T], BF16, kind="ExternalInput")
    xbf_d = nc.dram_tensor("xbf", [T, H], BF16, kind="ExternalInput")
    wg_d = nc.dram_tensor("wgt", [P, 2 * HC * E], BF16, kind="ExternalInput")
    w1_d = nc.dram_tensor("w1l", [EL, FT, P, HC * 2 * P], BF16,
                          kind="ExternalInput")
    w2_d = nc.dram_tensor("w2l", [EL, 4, P, FT * 512], BF16,
                          kind="ExternalInput")
    ws1_d = nc.dram_tensor("ws1l", [P, HC * 768], BF16, kind="ExternalInput")
    ws2_d = nc.dram_tensor("ws2l", [P, 3 * H], BF16, kind="ExternalInput")
    out_d = nc.dram_tensor("out", [P, H], BF16, kind="ExternalOutput")

    acc_d = [nc.dram_tensor(f"acc{i}", [T, H // 2], BF16) for i in range(2)]
    rs_d = [nc.dram_tensor(f"rs{i}", [P, H // 2], BF16) for i in range(2)]

    with tile.TileContext(nc) as tc:
        _build(nc, tc, xbfT_d, xrT_d, xbf_d, wg_d, w1_d, w2_d, ws1_d, ws2_d,
               out_d, acc_d, rs_d)

    nc.compile()
    return nc


def _build(nc, tc, xbfT_d, xrT_d, xbf_d, wg_d, w1_d, w2_d, ws1_d, ws2_d,
           out_d, acc_d, rs_d):
    import contextlib
    ctx = contextlib.ExitStack()
    sbA = ctx.enter_context(tc.tile_pool(name="sbA", bufs=1))
    ps = ctx.enter_context(tc.tile_pool(name="ps", bufs=1, space="PSUM"))

    def pA(name):
        return ps.tile([P, 512], F32, tag="pA", bufs=4, name=name)

    def pB(name, dt=F32):
        return ps.tile([P, C], dt, tag="pB", bufs=4, name=name)

    # ---- constants ----
    ident = sbA.tile([P, P], F32)
    make_identity(nc, ident[:])
    identb = sbA.tile([P, P], BF16)
    make_identity(nc, identb[:])
    iota_ci = sbA.tile([P, C], I32)
    nc.gpsimd.iota(iota_ci[:], pattern=[[1, C]], base=0, channel_multiplier=0)
    iota_c = sbA.tile([P, C], F32)
    nc.vector.tensor_copy(iota_c[:], iota_ci[:])
    ihalf_i = sbA.tile([P, 2], I32)      # col h: value 128*h + p
    nc.gpsimd.iota(ihalf_i[:], pattern=[[P, 2]], base=0, channel_multiplier=1)
    ihalf_bf = sbA.tile([P, 2], BF16)
    nc.vector.tensor_copy(ihalf_bf[:], ihalf_i[:])
    toki = sbA.tile([P, TT], I32)        # col k: value 128*k + p
    nc.gpsimd.iota(toki[:], pattern=[[P, TT]], base=0, channel_multiplier=1)
    tok5 = sbA.tile([P, TT * 5], F32R)   # per k: [tok | w_e0..w_e3]
    tok5v = tok5[:].rearrange("p (k c) -> p k c", c=5)
    nc.vector.tensor_copy(tok5v[:, :, 0:1], toki[:].rearrange(
        "p (k c) -> p k c", c=1))
    tri = sbA.tile([P, TRIW], BF16)      # tri[p, q] = (q >= p + 897)
    nc.vector.memset(tri[:], 1.0)
    nc.gpsimd.affine_select(out=tri[:], in_=tri[:], pattern=[[1, TRIW]],
                            compare_op=OP.is_ge, fill=0.0, base=-897,
                            channel_multiplier=-1)
    sel4 = sbA.tile([P, EL * P], BF16)   # [k, e*128+m] = (k == e)
    nc.vector.memset(sel4[:], 1.0)
    nc.gpsimd.affine_select(out=sel4[:], in_=sel4[:], pattern=[[1, EL * P]],
                            compare_op=OP.is_ge, fill=0.0, base=0,
                            channel_multiplier=-P)
    nc.gpsimd.affine_select(out=sel4[:], in_=sel4[:], pattern=[[-1, EL * P]],
                            compare_op=OP.is_ge, fill=0.0, base=P - 1,
                            channel_multiplier=P)

    # ---- persistent tiles ----
    act_sT = sbA.tile([P, 3 * T], BF16)          # shared act^T [fs, t]
    slotT_bf = sbA.tile([E, T], BF16)
    posTok = sbA.tile([P, TT * EL], F32)         # slot of token, local experts
    y_sb = [sbA.tile([P, 2 * H], BF16, tag=f"y{e}", name=f"y{e}")
            for e in range(EL)]
    gmat = [sbA.tile([P, 2 * T], BF16, tag=f"g{e}", name=f"gm{e}")
            for e in range(EL)]
    wslot = [sbA.tile([P, 2], F32, tag=f"w{e}", name=f"ws{e}")
             for e in range(EL)]
    xet = [sbA.tile([P, HC * C], BF16, tag=f"xet{e}", name=f"xet{e}")
           for e in range(EL)]

    # ================= phase 1: router + shared up-proj + routing =========
    with tc.tile_pool(name="sbB", bufs=1) as sbB:
        # Allocation order fixes the pool layout: early-dying tiles first so
        # phase-2's w1/w2 stream buffers (stack-reused addresses) only overlap
        # tiles that are dead by the end of expert prep; xbfT/ws1 (live until
        # the last shared-expert block) go last.
        wg_sb = sbB.tile([P, 2 * HC * E], BF16)   # [wg_bf | wg_res] tiled
        logT = sbB.tile([E, T], F32)
        for tg, sh, dt, nb in [("xr", [P, 2 * T], BF16, 2),
                               ("petk", [P, C], F32R, 2),
                               ("stokw", [E, C], F32, 2),
                               ("stoki", [P, 2], I32, 2),
                               ("xe", [P, H], BF16, 4),
                               ("srow", [P, T], BF16, 2),
                               ("gsil", [P, 512], F32, 2)]:
            sbB.tile(sh, dt, tag=tg, bufs=nb, name=f"touch_{tg}")
        scores = sbB.tile([P, TT * E], F32)
        comb = sbB.tile([P, TT * E], F32)
        mask_bf = sbB.tile([P, TT * E], BF16)
        pos = sbB.tile([E, T], F32)
        maskT = sbB.tile([E, T], BF16)
        tmp8 = sbB.tile([P, 8], F32)
        xbfT = sbB.tile([P, HC * T], BF16)
        ws1_sb = sbB.tile([P, HC * 768], BF16)

        nc.sync.dma_start(out=wg_sb[:], in_=wg_d[:, :])
        for q in range(8):
            w = HC * T // 8
            nc.scalar.dma_start(out=xbfT[:, q * w:(q + 1) * w],
                                in_=xbfT_d[:, q * w:(q + 1) * w])
        nc.scalar.dma_start(out=ws1_sb[:], in_=ws1_d[:, :])

        # router: logitsT = wg.T @ x.T in bf16 with residual correction:
        # wb.xb + wb.xr + wr.xb  (error ~1e-5, exceeds fp32-routing fidelity)
        psl = [ps.tile([E, T // 2], F32, tag="pA", bufs=4, name=f"psl{n}")
               for n in range(2)]
        for k in range(HC):
            if k % 2 == 0:
                xr = sbB.tile([P, 2 * T], BF16, tag="xr", bufs=2,
                              name=f"xr{k}")
                nc.sync.dma_start(out=xr[:],
                                  in_=xrT_d[:, k * T:(k + 2) * T])
            wb = wg_sb[:, k * E:(k + 1) * E]
            wr = wg_sb[:, (HC + k) * E:(HC + k + 1) * E]
            for n in range(2):
                xbk = xbfT[:, k * T + n * 512:k * T + (n + 1) * 512]
                xrk = xr[:, (k % 2) * T + n * 512:(k % 2) * T + (n + 1) * 512]
                nc.tensor.matmul(psl[n][:], wb, xbk,
                                 start=(k == 0), stop=False)
                nc.tensor.matmul(psl[n][:], wb, xrk, start=False, stop=False)
                nc.tensor.matmul(psl[n][:], wr, xbk, start=False,
                                 stop=(k == HC - 1))
        for n in range(2):
            nc.vector.tensor_copy(logT[:, n * (T // 2):(n + 1) * (T // 2)],
                                  psl[n][:])

        # transpose logitsT -> scores [128, (k e)]
        for k in range(TT):
            pst = pB(f"ltr{k}")
            nc.tensor.transpose(pst[:, :E], logT[:, k * P:(k + 1) * P],
                                ident[:E, :E])
            nc.vector.tensor_copy(scores[:, k * E:(k + 1) * E], pst[:, :E])

        # shared expert up-proj: act_sT[fs, t] (3 fs-tiles x 2 t-halves),
        # emitted in blocks interleaved with routing tail + expert prep so
        # the PE stays busy while routing/gather dependencies resolve
        def s1_block(mg):
            for n in range(2):
                psg = pA(f"psg{mg}{n}")
                psu = pA(f"psu{mg}{n}")
                for k in range(HC):
                    rhs = xbfT[:, k * T + n * 512:k * T + (n + 1) * 512]
                    nc.tensor.matmul(psg[:],
                                     ws1_sb[:, k * 768 + mg * P:
                                            k * 768 + (mg + 1) * P],
                                     rhs, start=(k == 0), stop=(k == HC - 1))
                    nc.tensor.matmul(psu[:],
                                     ws1_sb[:, k * 768 + SSH + mg * P:
                                            k * 768 + SSH + (mg + 1) * P],
                                     rhs, start=(k == 0), stop=(k == HC - 1))
                gsil = sbB.tile([P, 512], F32, tag="gsil", bufs=2,
                                name=f"gsil{mg}{n}")
                nc.scalar.activation(gsil[:], psg[:], AF.Sigmoid)
                nc.vector.tensor_tensor(out=gsil[:], in0=gsil[:], in1=psg[:],
                                        op=OP.mult)
                nc.vector.tensor_tensor(
                    out=act_sT[:, mg * T + n * 512:mg * T + (n + 1) * 512],
                    in0=gsil[:], in1=psu[:], op=OP.mult)

        s1_block(0)

        # softmax + grouped top-k (per t-tile)
        for k in range(TT):
            blk = scores[:, k * E:(k + 1) * E]
            mx = sbB.tile([P, 1], F32, tag="rmax", bufs=2, name=f"rmax{k}")
            nc.vector.tensor_reduce(mx[:], blk, axis=mybir.AxisListType.X,
                                    op=OP.max, negate=True)
            sm = sbB.tile([P, 1], F32, tag="rsum", bufs=2, name=f"rsum{k}")
            nc.scalar.activation(blk, blk, AF.Exp, bias=mx[:], accum_out=sm[:])
            rc = sbB.tile([P, 1], F32, tag="rrec", bufs=2, name=f"rrec{k}")
            nc.vector.reciprocal(rc[:], sm[:])
            nc.vector.tensor_scalar_mul(blk, blk, rc[:])

            blk3 = blk.rearrange("p (g f) -> p g f", f=4)
            gsc = sbB.tile([P, G_GRP], F32, tag="gsc", bufs=2, name=f"gsc{k}")
            nc.vector.tensor_reduce(gsc[:], blk3, axis=mybir.AxisListType.X,
                                    op=OP.max)
            nc.vector.max(out=tmp8[:], in_=gsc[:])
            nc.vector.memset(tmp8[:, TOPK_G:], 0.0)
            gz = sbB.tile([P, G_GRP], F32, tag="gz", bufs=2, name=f"gz{k}")
            nc.vector.match_replace(out=gz[:], in_to_replace=tmp8[:],
                                    in_values=gsc[:], imm_value=0.0)
            nc.vector.tensor_tensor(out=gz[:], in0=gsc[:], in1=gz[:],
                                    op=OP.subtract)
            nc.vector.tensor_scalar(gz[:], gz[:], 0.0, scalar2=None,
                                    op0=OP.is_gt)
            cblk = comb[:, k * E:(k + 1) * E]
            cblk3 = cblk.rearrange("p (g f) -> p g f", f=4)
            gz3 = gz[:].rearrange("p (g o) -> p g o", o=1)
            nc.vector.tensor_tensor(out=cblk3, in0=blk3,
                                    in1=gz3.to_broadcast([P, G_GRP, 4]),
                                    op=OP.mult)
            nc.vector.max(out=tmp8[:], in_=cblk)
            nc.vector.memset(tmp8[:, TOPK:], 0.0)
            zap = sbB.tile([P, E], F32, tag="zap", bufs=2, name=f"zap{k}")
            nc.vector.match_replace(out=zap[:], in_to_replace=tmp8[:],
                                    in_values=cblk, imm_value=0.0)
            nc.vector.tensor_tensor(out=cblk, in0=cblk, in1=zap[:],
                                    op=OP.subtract)
            nc.vector.tensor_scalar_mul(cblk, cblk, SCALE)
            nc.vector.tensor_scalar(mask_bf[:, k * E:(k + 1) * E], cblk, 0.0,
                                    scalar2=None, op0=OP.is_gt)
        # local-expert weights into the slot-matmul lhsT (cols 1..4 per k)
        nc.vector.tensor_copy(
            tok5v[:, :, 1:5],
            comb[:].rearrange("p (k e) -> p k e", e=E)[:, :, 0:EL])

        # strict cumsum: pos[e, t] = sum_{t' < t} mask[e, t']
        for n in range(2):
            psc = ps.tile([E, T // 2], F32, tag="pA", bufs=4, name=f"psc{n}")
            for k in range(TT):
                s = 896 - k * P + n * 512
                nc.tensor.matmul(psc[:], mask_bf[:, k * E:(k + 1) * E],
                                 tri[:, s:s + 512],
                                 start=(k == 0), stop=(k == TT - 1))
            nc.vector.tensor_copy(pos[:, n * (T // 2):(n + 1) * (T // 2)],
                                  psc[:])
        # maskT via PE transpose of mask_bf
        for k in range(TT):
            ptm = pB(f"mtr{k}", BF16)
            nc.tensor.transpose(ptm[:E, :P], mask_bf[:, k * E:(k + 1) * E],
                                identb[:])
            nc.vector.tensor_copy(maskT[:, k * P:(k + 1) * P], ptm[:E, :P])
        # slot[e,t] = mask ? pos : C   == C + (pos - C) * mask
        nc.vector.tensor_scalar(pos[:], pos[:], float(C), scalar2=None,
                                op0=OP.subtract)
        nc.vector.tensor_tensor(out=pos[:], in0=pos[:], in1=maskT[:],
                                op=OP.mult)
        nc.vector.tensor_scalar(pos[:], pos[:], float(C), scalar2=None,
                                op0=OP.add)
        nc.vector.tensor_copy(slotT_bf[:], pos[:])
        # posTok: slot values token-major for the local experts
        for k in range(TT):
            ptt = pB(f"ptr{k}")
            nc.tensor.transpose(ptt[:, :E], pos[:, k * P:(k + 1) * P],
                                ident[:E, :E])
            nc.vector.tensor_copy(posTok[:, k * EL:(k + 1) * EL],
                                  ptt[:, 0:EL])

        # ---- prep one expert: srow, slot->token, gather, transpose ----
        def prep(e):
            # srow: broadcast slotT row e to all partitions
            srow = sbB.tile([P, T], BF16, tag="srow", bufs=2, name=f"srow{e}")
            for n in range(2):
                psb = pA(f"srow{e}{n}")
                nc.tensor.matmul(psb[:], sel4[0:E, e * P:(e + 1) * P],
                                 slotT_bf[0:E, n * 512:(n + 1) * 512],
                                 start=True, stop=True)
                nc.vector.tensor_copy(srow[:, n * 512:(n + 1) * 512], psb[:])
            for half in range(2):
                nc.vector.tensor_tensor(
                    out=gmat[e][:, half * T:(half + 1) * T],
                    in0=ihalf_bf[:, half:half + 1].to_broadcast([P, T]),
                    in1=srow[:], op=OP.is_equal)
            # slot -> (token id, weight): accumulate over t-tiles
            pstk = ps.tile([5, C], F32, tag="pB", bufs=4, name=f"stk{e}")
            for k in range(TT):
                ptk = sbB.tile([P, C], F32R, tag="petk", bufs=2,
                               name=f"petk{e}{k}")
                nc.vector.tensor_tensor(
                    out=ptk[:],
                    in0=posTok[:, k * EL + e:k * EL + e + 1].to_broadcast(
                        [P, C]),
                    in1=iota_c[:], op=OP.is_equal)
                nc.tensor.matmul(pstk[:], tok5[:, k * 5:(k + 1) * 5], ptk[:],
                                 start=(k == 0), stop=(k == TT - 1))
            stokw = sbB.tile([E, C], F32, tag="stokw", bufs=2,
                             name=f"stokw{e}")
            nc.vector.memset(stokw[:], 0.0)
            nc.vector.tensor_copy(stokw[0:5, :], pstk[:])
            stok_i = sbB.tile([P, 2], I32, tag="stoki", bufs=2,
                              name=f"stoki{e}")
            for half in range(2):
                ptt = pB(f"st{e}{half}")
                nc.tensor.transpose(ptt[:, 0:E],
                                    stokw[0:E, half * P:(half + 1) * P],
                                    ident[0:E, 0:E])
                nc.vector.tensor_copy(stok_i[:, half:half + 1], ptt[:, 0:1])
                nc.vector.tensor_copy(wslot[e][:, half:half + 1],
                                      ptt[:, 1 + e:2 + e])
            # gather x rows, then XBAR DMA-transpose to [h, c] layout
            for half in range(2):
                xe = sbB.tile([P, H], BF16, tag="xe", bufs=4,
                              name=f"xe{e}{half}")
                nc.gpsimd.indirect_dma_start(
                    out=xe[:], out_offset=None, in_=xbf_d[:, :],
                    in_offset=bass.IndirectOffsetOnAxis(
                        ap=stok_i[:, half:half + 1], axis=0))
                outap = xet[e][:].rearrange("p (hc c) -> p hc c", hc=HC)[
                    :, :, half * P:half * P + P]
                nc.scalar.dma_start(out=outap, in_=xe[:], transpose=True)

        prep(0)
        prep(1)
        prep(2)
        prep(3)
        s1_block(1)
        s1_block(2)

    # ================= phase 2: expert MLPs ===============================
    with tc.tile_pool(name="sbC", bufs=1) as sbC:
        ws2_sb = sbC.tile([P, 3 * H], BF16)
        nc.scalar.dma_start(out=ws2_sb[:], in_=ws2_d[:, :])
        # ---- MM1 + MM2 per expert ----
        for e in range(EL):
            act_e = sbC.tile([P, FT * C], BF16, tag="act", bufs=2,
                             name=f"act{e}")
            for m in range(FT):
                w1p = sbC.tile([P, HC * 2 * P], BF16, tag="w1p", bufs=4,
                               name=f"w1p{e}{m}")
                nc.sync.dma_start(out=w1p[:], in_=w1_d[e, m, :, :])
                psg = pB(f"mg{e}{m}")
                psu = pB(f"mu{e}{m}")
                for k in range(HC):
                    rhs = xet[e][:, k * C:(k + 1) * C]
                    nc.tensor.matmul(psg[:], w1p[:, k * 2 * P:k * 2 * P + P],
                                     rhs, start=(k == 0), stop=(k == HC - 1))
                    nc.tensor.matmul(psu[:],
                                     w1p[:, k * 2 * P + P:(k + 1) * 2 * P],
                                     rhs, start=(k == 0), stop=(k == HC - 1))
                sgt = sbC.tile([P, C], F32, tag="sgt", bufs=2,
                               name=f"sgt{e}{m}")
                nc.scalar.activation(sgt[:], psg[:], AF.Sigmoid)
                nc.vector.tensor_tensor(out=sgt[:], in0=sgt[:], in1=psg[:],
                                        op=OP.mult)
                nc.vector.tensor_tensor(out=act_e[:, m * C:(m + 1) * C],
                                        in0=sgt[:], in1=psu[:], op=OP.mult)
            for n in range(4):
                w2t = sbC.tile([P, FT * 512], BF16, tag="w2t", bufs=2,
                               name=f"w2t{e}{n}")
                nc.sync.dma_start(out=w2t[:], in_=w2_d[e, n, :, :])
                psy = [pA(f"y{e}{n}{mc}") for mc in range(2)]
                for kf in range(FT):
                    for mc in range(2):
                        nc.tensor.matmul(
                            psy[mc][:],
                            act_e[:, kf * C + mc * P:kf * C + (mc + 1) * P],
                            w2t[:, kf * 512:(kf + 1) * 512],
                            start=(kf == 0), stop=(kf == FT - 1))
                for mc in range(2):
                    nc.vector.tensor_scalar_mul(
                        y_sb[e][:, mc * H + n * 512:mc * H + (n + 1) * 512],
                        psy[mc][:], wslot[e][:, mc:mc + 1])

        # ================= phase 3: combine + ReduceScatter ===============
        for hh in range(2):
            for mt in range(TT):
                for nn in range(2):
                    pc = pA(f"c{hh}{mt}{nn}")
                    col = hh * 1024 + nn * 512
                    for kf in range(3):
                        nc.tensor.matmul(
                            pc[:],
                            act_sT[:, kf * T + mt * P:kf * T + (mt + 1) * P],
                            ws2_sb[:, kf * H + col:kf * H + col + 512],
                            start=(kf == 0), stop=False)
                    cnt = 3
                    for e in range(EL):
                        for ch in range(2):
                            cnt += 1
                            nc.tensor.matmul(
                                pc[:],
                                gmat[e][:, ch * T + mt * P:
                                        ch * T + (mt + 1) * P],
                                y_sb[e][:, ch * H + col:ch * H + col + 512],
                                start=False, stop=(cnt == 11))
                    ob = sbC.tile([P, 512], BF16, tag="ob", bufs=4,
                                  name=f"ob{hh}{mt}{nn}")
                    nc.vector.tensor_copy(ob[:], pc[:])
                    nc.sync.dma_start(
                        out=acc_d[hh][mt * P:(mt + 1) * P,
                                      nn * 512:(nn + 1) * 512],
                        in_=ob[:])
            nc.gpsimd.collective_compute(
                "ReduceScatter", OP.add,
                replica_groups=[list(range(NCORES))],
                ins=[acc_d[hh][:, :]], outs=[rs_d[hh][:, :]])
            nc.scalar.dma_start(out=out_d[:, hh * 1024:(hh + 1) * 1024],
                                in_=rs_d[hh][:, :])

    ctx.close()


# ---------------- host side ----------------
_CACHED = {}


def _get_program():
    if "nc" not in _CACHED:
        _CACHED["nc"] = build_program()
    return _CACHED["nc"]


def make_in_maps(hidden_states, w_gate, w1, w2, ws1, ws2):
    bf = ml_dtypes.bfloat16
    x = np.ascontiguousarray(hidden_states, dtype=np.float32)
    xTt = np.ascontiguousarray(
        x.T.reshape(HC, P, T).transpose(1, 0, 2).reshape(P, HC * T))
    xbfT = xTt.astype(bf)
    xrT = (xTt - xbfT.astype(np.float32)).astype(bf)
    xbf = x.astype(bf)                                             # [T, H]
    wg = np.asarray(w_gate, np.float32)                            # [E, H]
    w1 = np.asarray(w1, np.float32)
    w2 = np.asarray(w2, np.float32)
    ws1 = np.asarray(ws1, np.float32)
    ws2 = np.asarray(ws2, np.float32)

    # w1 interleaved (gate_m | up_m) then tiled [E, FT, P, HC*256]
    w1c = np.concatenate([w1[:, :, :F].reshape(E, H, FT, P),
                          w1[:, :, F:].reshape(E, H, FT, P)], axis=3)
    w1t = np.ascontiguousarray(
        w1c.reshape(E, HC, P, FT, 2 * P).transpose(0, 3, 2, 1, 4).reshape(
            E, FT, P, HC * 2 * P)).astype(bf)
    # w2 tiled [E, 4, P, FT*512]
    w2t = np.ascontiguousarray(
        w2.reshape(E, FT, P, 4, 512).transpose(0, 3, 2, 1, 4).reshape(
            E, 4, P, FT * 512)).astype(bf)

    shard = FS // NCORES  # 352
    in_maps = []
    for k in range(NCORES):
        # shared expert shard, padded 352 -> 384, tiled [P, HC*768]
        ws1p = np.zeros((H, 2 * SSH), np.float32)
        ws1p[:, :shard] = ws1[:, k * shard:(k + 1) * shard]
        ws1p[:, SSH:SSH + shard] = ws1[:, FS + k * shard:FS + (k + 1) * shard]
        ws1l = np.ascontiguousarray(
            ws1p.reshape(HC, P, 2 * SSH).transpose(1, 0, 2).reshape(
                P, HC * 768)).astype(bf)
        ws2p = np.zeros((SSH, H), np.float32)
        ws2p[:shard] = ws2[k * shard:(k + 1) * shard]
        ws2l = np.ascontiguousarray(
            ws2p.reshape(3, P, H).transpose(1, 0, 2).reshape(
                P, 3 * H)).astype(bf)
        # group permutation: swap group 0 <-> group k so the core's own
        # 4 experts are rows 0..3 (grouped top-k is group-order invariant)
        gperm = list(range(G_GRP))
        gperm[0], gperm[k] = gperm[k], gperm[0]
        eperm = [g * EL + i for g in gperm for i in range(EL)]
        wgp = wg[eperm]                                            # [E, H]
        wgtf = np.ascontiguousarray(
            wgp.T.reshape(HC, P, E).transpose(1, 0, 2).reshape(P, HC * E))
        wgb = wgtf.astype(bf)
        wgr = (wgtf - wgb.astype(np.float32)).astype(bf)
        in_maps.append({
            "xbfT": xbfT,
            "xrT": xrT,
            "xbf": xbf,
            "wgt": np.ascontiguousarray(
                np.concatenate([wgb, wgr], axis=1)),
            "w1l": np.ascontiguousarray(w1t[k * EL:(k + 1) * EL]),
            "w2l": np.ascontiguousarray(w2t[k * EL:(k + 1) * EL]),
            "ws1l": ws1l,
            "ws2l": ws2l,
        })
    return in_maps


def kernel(hidden_states, w_gate, w1, w2, ws1, ws2):
    from concourse.bass_utils import run_bass_kernel_spmd
    nc = _get_program()
    in_maps = make_in_maps(hidden_states, w_gate, w1, w2, ws1, ws2)
    res = run_bass_kernel_spmd(nc, in_maps, list(range(NCORES)))
    shards = [res.results[k]["out"] for k in range(NCORES)]
    return np.concatenate(shards, axis=0).astype(np.float32)
